# revision 1
# baseline (speedup 1.0000x reference)
"""Gemma3 decoder layer on 8 Trainium2 NeuronCores (Bass/Tile).

Sharding (per core c):
  - attention: tensor-parallel over heads; core c owns Q head c, KV head c//2.
  - wo: replicated weight, token-sharded rows (core c computes tokens [256c, 256c+256)).
  - MLP: gate/up column-sharded (1280 cols each), down row-sharded; partial
    sums combined with a ReduceScatter that lands each core its token shard.
  - norms/residual: token-sharded.
Dataflow: in_ln -> AG(h^T) -> QKV+rope+attn -> A2A(attn^T) -> wo+norms ->
  AG(h2^T) -> gate/up/down -> RS -> post_ff+residual.
Matmuls run in bf16 (fp32 PSUM accumulation); norms/softmax/residual in fp32.
"""
import sys

if "/opt/trn_rl_repo" not in sys.path:
    sys.path.insert(0, "/opt/trn_rl_repo")

import numpy as np
import ml_dtypes

import concourse.bass as bass
import concourse.mybir as mybir
import concourse.tile as tile
from concourse import bacc
from concourse.bass_utils import run_bass_kernel_spmd
from concourse.masks import make_identity

dt = mybir.dt
AF = mybir.ActivationFunctionType
ALU = mybir.AluOpType
BF = dt.bfloat16
F32 = dt.float32

HID, NH, NKV, HD, INTER = 2560, 8, 4, 256, 10240
WIN, EPS, BASE = 512, 1e-6, 10000.0
S = 2048
NC_ = 8
TS = S // NC_              # 256 tokens per core
KH = HID // 128            # 20 hidden-dim chunks
KA = (NH * HD) // 128      # 16 attn-dim chunks
MI = INTER // NC_ // 128   # 10 inter m-tiles per core
HALF = HD // 2


def _bcast_row(nc, sbuf_tile, dram_t, width):
    a = dram_t.ap()
    nc.sync.dma_start(sbuf_tile[:], bass.AP(
        tensor=a.tensor, offset=a.offset, ap=[[0, 128], [1, width]]))


def _swap_ap(t, w):
    """Read tile t [128, 2w] with free-dim halves swapped (as [128,2,w])."""
    a = t[:, 0:2 * w]
    return bass.AP(tensor=a.tensor, offset=a.offset + w,
                   ap=[list(a.ap[0]), [-w, 2], [1, w]])


def build_nc(sim=False):
    nc = bacc.Bacc("TRN2", target_bir_lowering=False, debug=False,
                   enable_asserts=True, num_devices=1 if sim else NC_)

    def _coll(kind, op, ins, outs):
        if not sim:
            nc.gpsimd.collective_compute(kind, op, replica_groups=rg,
                                         ins=ins, outs=outs)
            return
        i_ap, o_ap = ins[0], outs[0]
        if kind == "AllGather":
            n = i_ap.shape[0]
            for r in range(NC_):
                nc.sync.dma_start(o_ap[r * n:(r + 1) * n], i_ap)
        elif kind == "AllToAll":
            nc.sync.dma_start(o_ap, i_ap)
        elif kind == "ReduceScatter":
            n = o_ap.shape[0]
            nc.sync.dma_start(o_ap, i_ap[0:n])

    x_shard = nc.dram_tensor("x_shard", [TS, HID], F32, kind="ExternalInput")
    wq_c = nc.dram_tensor("wq_c", [HID, HD], BF, kind="ExternalInput")
    wk_c = nc.dram_tensor("wk_c", [HID, HD], BF, kind="ExternalInput")
    wv_c = nc.dram_tensor("wv_c", [HID, HD], BF, kind="ExternalInput")
    wo_f = nc.dram_tensor("wo_f", [NH * HD, HID], BF, kind="ExternalInput")
    wg_c = nc.dram_tensor("wg_c", [HID, INTER // NC_], BF, kind="ExternalInput")
    wu_c = nc.dram_tensor("wu_c", [HID, INTER // NC_], BF, kind="ExternalInput")
    wd_c = nc.dram_tensor("wd_c", [INTER // NC_, HID], BF, kind="ExternalInput")
    w1_in = nc.dram_tensor("w1_in", [HID], BF, kind="ExternalInput")
    w1_pa = nc.dram_tensor("w1_pa", [HID], BF, kind="ExternalInput")
    w1_pf = nc.dram_tensor("w1_pf", [HID], BF, kind="ExternalInput")
    w1_po = nc.dram_tensor("w1_po", [HID], F32, kind="ExternalInput")
    cqw = nc.dram_tensor("cqw", [S, HD], BF, kind="ExternalInput")
    sqw = nc.dram_tensor("sqw", [S, HD], BF, kind="ExternalInput")
    ckw = nc.dram_tensor("ckw", [S, HD], BF, kind="ExternalInput")
    skw = nc.dram_tensor("skw", [S, HD], BF, kind="ExternalInput")
    out_shard = nc.dram_tensor("out_shard", [TS, HID], F32, kind="ExternalOutput")

    rg = [list(range(NC_))]
    stages = {}
    nc._stage_ids = stages

    def mark(name):
        stages[name] = nc.next_id()

    with tile.TileContext(nc) as tc:
        with (
            tc.tile_pool(name="dram", bufs=1, space="DRAM") as dram,
            tc.tile_pool(name="glob", bufs=1) as glob,
            tc.tile_pool(name="nrm", bufs=3) as nrm,
            tc.tile_pool(name="psP", bufs=1, space="PSUM") as psP,
        ):
            # DRAM scratch
            hT_in = dram.tile([HID, TS], BF)
            hT_full = dram.tile([NC_ * HID, TS], BF, addr_space="Local" if sim else "Shared")
            a2a_in = dram.tile([S, TS], BF)
            a2a_out = dram.tile([S, TS], BF)
            h2T_in = dram.tile([HID, TS], BF)
            h2T_full = dram.tile([NC_ * HID, TS], BF, addr_space="Local" if sim else "Shared")
            rs_in = dram.tile([S, HID], BF)
            rs_out = dram.tile([TS, HID], BF)
            x2_spill = dram.tile([TS, HID], F32)

            ident = glob.tile([128, 128], BF)
            make_identity(nc, ident[:])
            eps_t = glob.tile([128, 1], F32)
            nc.vector.memset(eps_t[:], EPS)

            def rmsnorm_rinv(src_ap, d, name):
                """rinv[p,1]=1/sqrt(mean(src^2)+EPS) via bn_stats + ln/exp."""
                nsub = max(1, d // 512)
                stats = nrm.tile([128, nsub, 6], F32, tag="nst", name=f"{name}_st")
                if nsub > 1:
                    view = src_ap.rearrange("p (s f) -> p s f", s=nsub)
                    for i in range(nsub):
                        nc.vector.bn_stats(out=stats[:, i, :], in_=view[:, i, :])
                else:
                    nc.vector.bn_stats(out=stats[:, 0, :], in_=src_ap)
                mv = nrm.tile([128, 2], F32, tag="nmv", name=f"{name}_mv")
                nc.vector.bn_aggr(out=mv[:], in_=stats[:])
                ms = nrm.tile([128, 1], F32, tag="nms", name=f"{name}_ms")
                nc.vector.scalar_tensor_tensor(ms[:], mv[:, 0:1], mv[:, 0:1],
                                               mv[:, 1:2], op0=ALU.mult, op1=ALU.add)
                lnm = nrm.tile([128, 1], F32, tag="nln", name=f"{name}_ln")
                nc.scalar.activation(lnm[:], ms[:], AF.Ln, bias=eps_t[:])
                rinv = nrm.tile([128, 1], F32, tag="nrv", name=f"{name}_rv")
                nc.scalar.activation(rinv[:], lnm[:], AF.Exp, scale=-0.5)
                return rinv

            with tc.tile_pool(name="xpool", bufs=1) as xpool:
                x_sb = [xpool.tile([128, HID], F32, name=f"xt{t}") for t in range(2)]
                h16s = [None, None]

                mark('S1')
                # ============ S1: in_ln + transpose + AG1 ============
                with tc.tile_pool(name="s1", bufs=2) as s1:
                    w1_in_b = s1.tile([128, HID], BF, bufs=1)
                    _bcast_row(nc, w1_in_b, w1_in, HID)
                    for t in range(2):
                        nc.sync.dma_start(x_sb[t][:],
                                          x_shard.ap()[t * 128:(t + 1) * 128, :])
                        rinv = rmsnorm_rinv(x_sb[t][:], HID, f"inln{t}")
                        h16 = s1.tile([128, HID], BF, tag="h16", name=f"h16_{t}", bufs=2)
                        for cch in range(5):
                            sl = slice(cch * 512, (cch + 1) * 512)
                            nc.vector.scalar_tensor_tensor(h16[:, sl], x_sb[t][:, sl],
                                                           rinv[:], w1_in_b[:, sl],
                                                           op0=ALU.mult, op1=ALU.mult)
                        h16s[t] = h16
                    for k in range(KH):
                        hTk = s1.tile([128, TS], BF, tag="hTk", name=f"hTk{k}", bufs=3)
                        for t in range(2):
                            ptr = psP.tile([128, 128], BF, tag="tr", bufs=2,
                                           name=f"s1tr{k}_{t}")
                            nc.tensor.transpose(
                                ptr[:], h16s[t][:, k * 128:(k + 1) * 128], ident[:])
                            nc.vector.tensor_copy(hTk[:, t * 128:(t + 1) * 128], ptr[:])
                        nc.sync.dma_start(hT_in[k * 128:(k + 1) * 128, :], hTk[:])
                    _coll("AllGather", ALU.bypass, [hT_in[:]], [hT_full[:]])

                mark('S2')
                # ============ S2/S3: attention ============
                wpool_cm = tc.tile_pool(name="wpool", bufs=1)
                wpool = wpool_cm.__enter__()
                wo_sb = wpool.tile([128, KA, HID], BF)
                nc.sync.dma_start(wo_sb[:],
                                  wo_f.ap().rearrange("(k p) n -> p k n", p=128))
                with tc.tile_pool(name="attp", bufs=1) as attp:
                    QTm = attp.tile([128, 2, S], BF, name="QTm")
                    KTm = attp.tile([128, 2, S], BF, name="KTm")
                    V = [attp.tile([128, HD + 1], BF, name=f"V{i}")
                         for i in range(S // 128)]
                    for i in range(S // 128):
                        nc.vector.memset(V[i][:, HD:HD + 1], 1.0)
                    aTm = attp.tile([128, 2, S], BF, name="aTm")
                    masks = attp.tile([128, 8, 512], BF)
                    for i in range(8):
                        delta = 512 - 128 * i
                        mk = masks[:, i, :]
                        nc.gpsimd.memset(mk, 1.0)
                        nc.gpsimd.affine_select(
                            out=mk, in_=mk, compare_op=ALU.is_ge, fill=0.0,
                            base=delta, pattern=[[1, 512]], channel_multiplier=-1)
                        nc.gpsimd.affine_select(
                            out=mk, in_=mk, compare_op=ALU.is_ge, fill=0.0,
                            base=-delta + (WIN - 1), pattern=[[-1, 512]],
                            channel_multiplier=1)
                    hT_v = hT_full[:].rearrange("(r k p) t -> r p k t", r=NC_, p=128)

                    with tc.tile_pool(name="s2", bufs=2) as s2:
                        wqk_sb = s2.tile([128, KH, 2 * HD], BF, bufs=1)
                        wv_sb = s2.tile([128, KH, HD], BF, bufs=1)
                        nc.sync.dma_start(wqk_sb[:, :, 0:HD],
                                          wq_c.ap().rearrange("(k p) n -> p k n", p=128))
                        nc.sync.dma_start(wqk_sb[:, :, HD:2 * HD],
                                          wk_c.ap().rearrange("(k p) n -> p k n", p=128))
                        nc.sync.dma_start(wv_sb[:],
                                          wv_c.ap().rearrange("(k p) n -> p k n", p=128))
                        for rb in range(NC_):
                            hTrb = s2.tile([128, KH, TS], BF, tag="hTrb",
                                           name=f"hTrb{rb}", bufs=2)
                            nc.sync.dma_start(hTrb[:], hT_v[rb])
                            for u in range(2):
                                tt = rb * 2 + u
                                pqk = psP.tile([128, 2 * HD], F32, tag="mm", bufs=6,
                                               name=f"pqk{tt}")
                                pv = psP.tile([128, HD], F32, tag="mm", bufs=6,
                                              name=f"pv{tt}")
                                for k in range(KH):
                                    st, sp = (k == 0), (k == KH - 1)
                                    lh = hTrb[:, k, u * 128:(u + 1) * 128]
                                    nc.tensor.matmul(pqk[:], lh, wqk_sb[:, k, :],
                                                     start=st, stop=sp)
                                    nc.tensor.matmul(pv[:], lh, wv_sb[:, k, :],
                                                     start=st, stop=sp)
                                nc.scalar.activation(V[tt][:, 0:HD], pv[:], AF.Copy)
                                for (qo, tab_c, tab_s, QKT, nm) in (
                                        (0, cqw, sqw, QTm, "q"),
                                        (HD, ckw, skw, KTm, "k")):
                                    srcp = pqk[:, qo:qo + HD]
                                    swp = bass.AP(tensor=srcp.tensor,
                                                  offset=srcp.offset + HALF,
                                                  ap=[list(srcp.ap[0]),
                                                      [-HALF, 2], [1, HALF]])
                                    rinv = rmsnorm_rinv(srcp, HD, f"{nm}n{tt}")
                                    ct = s2.tile([128, HD], BF, tag=f"c{nm}",
                                                 name=f"c{nm}{tt}", bufs=3)
                                    st_ = s2.tile([128, HD], BF, tag=f"s{nm}",
                                                  name=f"s{nm}{tt}", bufs=3)
                                    nc.sync.dma_start(
                                        ct[:], tab_c.ap()[tt * 128:(tt + 1) * 128, :])
                                    nc.sync.dma_start(
                                        st_[:], tab_s.ap()[tt * 128:(tt + 1) * 128, :])
                                    t1 = s2.tile([128, HD], BF, tag="t1",
                                                 name=f"t1{nm}{tt}", bufs=3)
                                    t2 = s2.tile([128, HD], BF, tag="t2",
                                                 name=f"t2{nm}{tt}", bufs=3)
                                    nc.vector.scalar_tensor_tensor(
                                        t1[:], srcp, rinv[:], ct[:],
                                        op0=ALU.mult, op1=ALU.mult)
                                    nc.vector.scalar_tensor_tensor(
                                        t2[:].rearrange("p (a b) -> p a b", a=2),
                                        swp, rinv[:],
                                        st_[:].rearrange("p (a b) -> p a b", a=2),
                                        op0=ALU.mult, op1=ALU.mult)
                                    qr = s2.tile([128, HD], BF, tag="qr",
                                                 name=f"qr{nm}{tt}", bufs=3)
                                    nc.vector.tensor_add(qr[:], t1[:], t2[:])
                                    ptr = psP.tile([128, HD], BF, tag="tr",
                                                   bufs=2, name=f"s2t{nm}{tt}")
                                    for h in range(2):
                                        nc.tensor.transpose(
                                            ptr[:, h * 128:(h + 1) * 128],
                                            qr[:, h * 128:(h + 1) * 128], ident[:])
                                    nc.vector.tensor_copy(
                                        QKT[:, :, tt * 128:(tt + 1) * 128],
                                        ptr[:].rearrange("p (a b) -> p a b", a=2))

                    mark('S3')
                    with tc.tile_pool(name="s3", bufs=2) as s3:
                        for qb in range(4):
                            q0 = 512 * qb
                            probs = {}
                            for i in range(8):
                                kc = q0 - 512 + 128 * i
                                if kc < 0:
                                    continue
                                psc = psP.tile([128, 512], F32, tag="mm", bufs=6,
                                               name=f"psc{qb}_{i}")
                                for h in range(2):
                                    nc.tensor.matmul(psc[:], KTm[:, h, kc:kc + 128],
                                                     QTm[:, h, q0:q0 + 512],
                                                     start=(h == 0), stop=(h == 1))
                                pr = s3.tile([128, 512], BF, tag="pr",
                                             name=f"pr{qb}_{i}", bufs=10)
                                nc.scalar.activation(pr[:], psc[:], AF.Exp,
                                                     scale=1.0 / 16.0)
                                nc.vector.tensor_mul(pr[:], pr[:], masks[:, i, :])
                                probs[kc] = pr
                            for qs in range(4):
                                qa = q0 + 128 * qs
                                kcs = [kc for kc in range(qa - 512, qa + 128, 128)
                                       if kc >= 0]
                                po = psP.tile([128, HD + 1], F32, tag="mm", bufs=6,
                                              name=f"po{qb}_{qs}")
                                col = qa - q0
                                for j, kc in enumerate(kcs):
                                    nc.tensor.matmul(po[:],
                                                     probs[kc][:, col:col + 128],
                                                     V[kc // 128][:], start=(j == 0),
                                                     stop=(j == len(kcs) - 1))
                                rec = s3.tile([128, 1], F32, tag="rec",
                                              name=f"rec{qb}_{qs}")
                                nc.vector.reciprocal(rec[:], po[:, HD:HD + 1])
                                an = s3.tile([128, HD], BF, tag="an",
                                             name=f"an{qb}_{qs}")
                                nc.vector.tensor_scalar_mul(an[:], po[:, 0:HD], rec[:])
                                ptr = psP.tile([128, HD], BF, tag="tr", bufs=2,
                                               name=f"s3tr{qb}{qs}")
                                for h in range(2):
                                    nc.tensor.transpose(
                                        ptr[:, h * 128:(h + 1) * 128],
                                        an[:, h * 128:(h + 1) * 128], ident[:])
                                nc.vector.tensor_copy(
                                    aTm[:, :, qa:qa + 128],
                                    ptr[:].rearrange("p (a b) -> p a b", a=2))
                        # A2A: input shard j = my head's attn^T for token block j;
                        # out rows [256i:256(i+1)) = head i's attn^T for my block.
                        for j in range(NC_):
                            for h in range(2):
                                nc.sync.dma_start(
                                    a2a_in[TS * j + 128 * h:TS * j + 128 * (h + 1), :],
                                    aTm[:, h, TS * j:TS * (j + 1)])
                        _coll("AllToAll", ALU.bypass, [a2a_in[:]], [a2a_out[:]])

                mark('S4')
                # ======== S4: wo + post_attn + residual + pre_ff + AG3 ========
                if True:
                    with tc.tile_pool(name="s4", bufs=2) as s4:
                        w1_pa_b = s4.tile([128, HID], BF, bufs=1)
                        w1_pf_b = s4.tile([128, HID], BF, bufs=1)
                        _bcast_row(nc, w1_pa_b, w1_pa, HID)
                        _bcast_row(nc, w1_pf_b, w1_pf, HID)
                        asl_v = a2a_out[:].rearrange("(k p) t -> p k t", p=128)
                        for t in range(2):
                            asl = s4.tile([128, KA, 128], BF, tag="asl",
                                          name=f"asl{t}", bufs=2)
                            nc.sync.dma_start(asl[:],
                                              asl_v[:, :, t * 128:(t + 1) * 128])
                            ao32 = s4.tile([128, HID], F32, tag="ao32",
                                           name=f"ao32_{t}", bufs=2)
                            for n in range(5):
                                pw = psP.tile([128, 512], F32, tag="mm", bufs=6,
                                              name=f"pw{t}_{n}")
                                for k in range(KA):
                                    nc.tensor.matmul(
                                        pw[:], asl[:, k, :],
                                        wo_sb[:, k, n * 512:(n + 1) * 512],
                                        start=(k == 0), stop=(k == KA - 1))
                                nc.vector.tensor_copy(ao32[:, n * 512:(n + 1) * 512],
                                                      pw[:])
                            rinv_a = rmsnorm_rinv(ao32[:], HID, f"pan{t}")
                            x2 = s4.tile([128, HID], F32, tag="x2", name=f"x2_{t}",
                                         bufs=2)
                            nc.vector.scalar_tensor_tensor(
                                x2[:], ao32[:], rinv_a[:], w1_pa_b[:],
                                op0=ALU.mult, op1=ALU.mult)
                            nc.vector.tensor_add(x2[:], x2[:], x_sb[t][:])
                            nc.sync.dma_start(x2_spill[t * 128:(t + 1) * 128, :], x2[:])
                            rinv_f = rmsnorm_rinv(x2[:], HID, f"pff{t}")
                            h2 = s4.tile([128, HID], BF, tag="h2", name=f"h2_{t}",
                                         bufs=2)
                            nc.vector.scalar_tensor_tensor(
                                h2[:], x2[:], rinv_f[:], w1_pf_b[:],
                                op0=ALU.mult, op1=ALU.mult)
                            h16s[t] = h2
                        for k in range(KH):
                            hTk = s4.tile([128, TS], BF, tag="hTk2",
                                          name=f"hTk2_{k}", bufs=3)
                            for t in range(2):
                                ptr = psP.tile([128, 128], BF, tag="tr", bufs=2,
                                               name=f"s4tr{k}_{t}")
                                nc.tensor.transpose(
                                    ptr[:], h16s[t][:, k * 128:(k + 1) * 128],
                                    ident[:])
                                nc.vector.tensor_copy(hTk[:, t * 128:(t + 1) * 128],
                                                      ptr[:])
                            nc.sync.dma_start(h2T_in[k * 128:(k + 1) * 128, :], hTk[:])
                        _coll("AllGather", ALU.bypass, [h2T_in[:]], [h2T_full[:]])
                wpool_cm.__exit__(None, None, None)

            mark('S5')
            # ================= S5: MLP =================
            with tc.tile_pool(name="s5w", bufs=1) as s5w:
                h2T_sb = s5w.tile([128, KH, NC_, TS], BF)
                h2T_fv = h2T_full[:].rearrange("(r k p) t -> r p k t", r=NC_, p=128)
                for r in range(NC_):
                    nc.sync.dma_start(h2T_sb[:, :, r, :], h2T_fv[r])
                actT = [s5w.tile([128, S], BF, name=f"actT{m}") for m in range(MI)]
                with tc.tile_pool(name="s5", bufs=2) as s5:
                    for m in range(MI):
                        wgm = s5.tile([128, KH, 128], BF, tag="wgm",
                                      name=f"wgm{m}", bufs=2)
                        wum = s5.tile([128, KH, 128], BF, tag="wum",
                                      name=f"wum{m}", bufs=2)
                        nc.sync.dma_start(wgm[:], wg_c.ap()[:, m * 128:(m + 1) * 128]
                                          .rearrange("(k p) n -> p k n", p=128))
                        nc.sync.dma_start(wum[:], wu_c.ap()[:, m * 128:(m + 1) * 128]
                                          .rearrange("(k p) n -> p k n", p=128))
                        for r in range(0, NC_, 2):
                            pg = psP.tile([128, 2 * TS], F32, tag="mm", bufs=6,
                                          name=f"pg{m}_{r}")
                            pu = psP.tile([128, 2 * TS], F32, tag="mm", bufs=6,
                                          name=f"pu{m}_{r}")
                            pg3 = pg[:].rearrange("p (a b) -> p a b", a=2)
                            pu3 = pu[:].rearrange("p (a b) -> p a b", a=2)
                            for k in range(KH):
                                st, sp = (k == 0), (k == KH - 1)
                                nc.tensor.matmul(pg3, wgm[:, k, :],
                                                 h2T_sb[:, k, r:r + 2, :],
                                                 start=st, stop=sp)
                                nc.tensor.matmul(pu3, wum[:, k, :],
                                                 h2T_sb[:, k, r:r + 2, :],
                                                 start=st, stop=sp)
                            gsc = s5.tile([128, 2 * TS], F32, tag="gsc",
                                          name=f"gsc{m}_{r}", bufs=3)
                            nc.scalar.activation(gsc[:], pg[:], AF.Gelu_apprx_tanh)
                            nc.vector.tensor_mul(actT[m][:, r * TS:(r + 2) * TS],
                                                 gsc[:], pu[:])
                    for n in range(5):
                        wdn = s5.tile([128, MI, 512], BF, tag="wdn",
                                      name=f"wdn{n}", bufs=2)
                        nc.sync.dma_start(wdn[:], wd_c.ap()[:, n * 512:(n + 1) * 512]
                                          .rearrange("(i p) n -> p i n", p=128))
                        for tt in range(S // 128):
                            pd = psP.tile([128, 512], F32, tag="mm", bufs=6,
                                          name=f"pd{n}_{tt}")
                            for i in range(MI):
                                nc.tensor.matmul(pd[:],
                                                 actT[i][:, tt * 128:(tt + 1) * 128],
                                                 wdn[:, i, :], start=(i == 0),
                                                 stop=(i == MI - 1))
                            dcp = s5.tile([128, 512], BF, tag="dcp",
                                          name=f"dcp{n}_{tt}", bufs=4)
                            if tt % 2 == 0:
                                nc.vector.tensor_copy(dcp[:], pd[:])
                            else:
                                nc.scalar.activation(dcp[:], pd[:], AF.Copy)
                            nc.sync.dma_start(
                                rs_in[tt * 128:(tt + 1) * 128,
                                      n * 512:(n + 1) * 512], dcp[:])
                    _coll("ReduceScatter", ALU.add, [rs_in[:]], [rs_out[:]])

            mark('S6')
            # ============ S6: post_ff norm + residual ============
            with tc.tile_pool(name="s6", bufs=2) as s6:
                w1_po_b = s6.tile([128, HID], F32, bufs=1)
                _bcast_row(nc, w1_po_b, w1_po, HID)
                for t in range(2):
                    mlp16 = s6.tile([128, HID], BF, tag="mlp", name=f"mlp{t}", bufs=2)
                    nc.sync.dma_start(mlp16[:], rs_out[:][t * 128:(t + 1) * 128, :])
                    x2l = s6.tile([128, HID], F32, tag="x2l", name=f"x2l{t}", bufs=2)
                    nc.sync.dma_start(x2l[:], x2_spill[t * 128:(t + 1) * 128, :])
                    rinv_o = rmsnorm_rinv(mlp16[:], HID, f"pon{t}")
                    o32 = s6.tile([128, HID], F32, tag="o32", name=f"o32_{t}", bufs=2)
                    nc.vector.scalar_tensor_tensor(o32[:], mlp16[:], rinv_o[:],
                                                   w1_po_b[:], op0=ALU.mult,
                                                   op1=ALU.mult)
                    nc.vector.tensor_add(o32[:], o32[:], x2l[:])
                    nc.sync.dma_start(out_shard.ap()[t * 128:(t + 1) * 128, :], o32[:])

    nc.compile()
    return nc


_NC_CACHE = None


def _get_nc():
    global _NC_CACHE
    if _NC_CACHE is None:
        _NC_CACHE = build_nc()
    return _NC_CACHE


def make_in_maps(hidden_states, position_ids, wq, wk, wv, wo, q_ln_w, k_ln_w,
                 in_ln_w, post_attn_ln_w, pre_ff_ln_w, post_ff_ln_w,
                 w_gate, w_up, w_down):
    bf16 = ml_dtypes.bfloat16
    f32 = np.float32
    x = np.asarray(hidden_states, f32).reshape(S, HID)
    pos = np.asarray(position_ids).reshape(S).astype(np.float64)

    inv_freq = 1.0 / (BASE ** (np.arange(0, HD, 2, dtype=np.float64) / HD))
    freqs = pos[:, None] * inv_freq[None, :]
    emb = np.concatenate([freqs, freqs], axis=1)
    cos = np.cos(emb).astype(f32)
    sin = np.sin(emb).astype(f32)
    w1q = 1.0 + np.asarray(q_ln_w, f32)
    w1k = 1.0 + np.asarray(k_ln_w, f32)

    def rope_tabs(w1):
        w1sw = np.concatenate([w1[HALF:], w1[:HALF]])
        sgn = np.concatenate([-np.ones(HALF, f32), np.ones(HALF, f32)])
        return ((cos * w1[None, :]).astype(bf16),
                (sin * (w1sw * sgn)[None, :]).astype(bf16))

    cqw_np, sqw_np = rope_tabs(w1q)
    ckw_np, skw_np = rope_tabs(w1k)

    wq_r = np.asarray(wq, f32).reshape(HID, NH, HD)
    wk_r = np.asarray(wk, f32).reshape(HID, NKV, HD)
    wv_r = np.asarray(wv, f32).reshape(HID, NKV, HD)
    wg_r = np.asarray(w_gate, f32).reshape(HID, NC_, INTER // NC_)
    wu_r = np.asarray(w_up, f32).reshape(HID, NC_, INTER // NC_)
    wd_r = np.asarray(w_down, f32).reshape(NC_, INTER // NC_, HID)

    common = {
        "wo_f": np.asarray(wo, f32).astype(bf16),
        "w1_in": (1.0 + np.asarray(in_ln_w, f32)).astype(bf16),
        "w1_pa": (1.0 + np.asarray(post_attn_ln_w, f32)).astype(bf16),
        "w1_pf": (1.0 + np.asarray(pre_ff_ln_w, f32)).astype(bf16),
        "w1_po": 1.0 + np.asarray(post_ff_ln_w, f32),
        "cqw": cqw_np, "sqw": sqw_np, "ckw": ckw_np, "skw": skw_np,
    }
    in_maps = []
    for c in range(NC_):
        g = c // (NH // NKV)
        in_maps.append({
            "x_shard": np.ascontiguousarray(x[c * TS:(c + 1) * TS]),
            "wq_c": np.ascontiguousarray(wq_r[:, c, :]).astype(bf16),
            "wk_c": np.ascontiguousarray(wk_r[:, g, :]).astype(bf16),
            "wv_c": np.ascontiguousarray(wv_r[:, g, :]).astype(bf16),
            "wg_c": np.ascontiguousarray(wg_r[:, c, :]).astype(bf16),
            "wu_c": np.ascontiguousarray(wu_r[:, c, :]).astype(bf16),
            "wd_c": np.ascontiguousarray(wd_r[c]).astype(bf16),
            **common,
        })
    return in_maps


def kernel(**inputs):
    in_maps = make_in_maps(**inputs)
    nc = _get_nc()
    res = run_bass_kernel_spmd(nc, in_maps, core_ids=list(range(NC_)))
    out = np.concatenate([res.results[c]["out_shard"] for c in range(NC_)], axis=0)
    return out.reshape(1, S, HID).astype(np.float32)



# revision 11
# speedup vs baseline: 1.1369x; 1.1369x over previous
"""Gemma3 decoder layer on 8 Trainium2 NeuronCores (Bass/Tile), v2.

Sharding / dataflow (per core c, tokens [256c, 256c+256)):
  S1  in_ln locally; quantize h to fp8 hi+lo pair; transpose -> hT8.
  S2  QKV for ALL heads of MY tokens via compensated-fp8 DoubleRow matmuls
      (3 DR matmuls per 256-wide hid pair-chunk: hi*hi both chunks, plus the
      two hi*lo cross terms; ~bf16 accuracy at 0.75x bf16 PE cost under the
      DoubleRow perf model). Per-head q/k rmsnorm + rope applied locally in
      [tok, dim] layout, then transposed and shipped per-head via AllToAll.
  S3  per-head sliding-window attention in bf16 (accuracy-critical);
      attn^T shipped back to token owners via a second AllToAll.
  S4  wo in bf16 (streamed weight), post_attn norm + residual + pre_ff norm;
      h2 quantized to fp8 hi+lo and transposed -> h2T8.
  S5  MLP fully data-parallel over tokens: every core streams the FULL
      gate/up/down weights as fp8 hi+lo pairs (same bytes as bf16) and runs
      compensated DoubleRow matmuls. No AllGather / ReduceScatter at all.
  S6  post_ff norm + residual locally.
Weights are pre-scaled by 256 on the host; the scale is absorbed by the
rmsnorms (scale-invariant) or folded into activation scales (gelu, copies).
"""
import sys

if "/opt/trn_rl_repo" not in sys.path:
    sys.path.insert(0, "/opt/trn_rl_repo")

import numpy as np
import ml_dtypes

import concourse.bass as bass
import concourse.mybir as mybir
import concourse.tile as tile
from concourse import bacc
from concourse.bass_utils import run_bass_kernel_spmd
from concourse.masks import make_identity

dt = mybir.dt
AF = mybir.ActivationFunctionType
ALU = mybir.AluOpType
BF = dt.bfloat16
F32 = dt.float32
F8 = dt.float8e4
DR = mybir.MatmulPerfMode.DoubleRow

HID, NH, NKV, HD, INTER = 2560, 8, 4, 256, 10240
WIN, EPS, BASE = 512, 1e-6, 10000.0
S = 2048
NC_ = 8
TS = S // NC_              # 256 tokens per core
KH = HID // 128            # 20 hid 128-chunks
KP = KH // 2               # 10 hid pair-chunks (256 contraction per DR)
ND = (NH + NKV + NKV)      # 16 qkv dim chunks of 256 (8 q | 4 k | 4 v)
MT = INTER // 128          # 80 inter tiles
JP = MT // 2               # 40 inter pair-chunks
NT = HID // 128            # 20 hid output tiles
HALF = HD // 2
WSC = 256.0                # host-side weight scale (power of two)


def _bcast_row(nc, sbuf_tile, dram_t, width):
    a = dram_t.ap()
    nc.sync.dma_start(sbuf_tile[:], bass.AP(
        tensor=a.tensor, offset=a.offset, ap=[[0, 128], [1, width]]))


def _lohi(t_ap, n):
    """Given ap [...,2,n] positioned at (hi,lo), return [128, 2, n] iterating
    (lo, hi)."""
    a = t_ap
    return bass.AP(tensor=a.tensor, offset=a.offset + n,
                   ap=[list(a.ap[0]), [-n, 2], [1, n]])


def build_nc(sim=False):
    nc = bacc.Bacc("TRN2", target_bir_lowering=False, debug=False,
                   enable_asserts=True, num_devices=1 if sim else NC_)

    def _coll(kind, op, ins, outs):
        if not sim:
            nc.gpsimd.collective_compute(kind, op, replica_groups=rg,
                                         ins=ins, outs=outs)
            return
        i_ap, o_ap = ins[0], outs[0]
        if kind == "AllToAll":
            nc.sync.dma_start(o_ap, i_ap)

    x_shard = nc.dram_tensor("x_shard", [TS, HID], F32, kind="ExternalInput")
    wqkv8 = nc.dram_tensor("wqkv8", [ND * 128, KH * 2 * 256], F8,
                           kind="ExternalInput")
    wo16 = nc.dram_tensor("wo16", [NH * HD, HID], BF, kind="ExternalInput")
    wg8 = nc.dram_tensor("wg8", [MT * 128, KH * 2 * 128], F8,
                         kind="ExternalInput")
    wu8 = nc.dram_tensor("wu8", [MT * 128, KH * 2 * 128], F8,
                         kind="ExternalInput")
    wd8 = nc.dram_tensor("wd8", [NT * 128, MT * 2 * 128], F8,
                         kind="ExternalInput")
    w1_in = nc.dram_tensor("w1_in", [HID], BF, kind="ExternalInput")
    w1_pa = nc.dram_tensor("w1_pa", [HID], BF, kind="ExternalInput")
    w1_pf = nc.dram_tensor("w1_pf", [HID], BF, kind="ExternalInput")
    w1_po = nc.dram_tensor("w1_po", [HID], F32, kind="ExternalInput")
    cqw = nc.dram_tensor("cqw", [TS, HD], BF, kind="ExternalInput")
    sqw = nc.dram_tensor("sqw", [TS, HD], BF, kind="ExternalInput")
    ckw = nc.dram_tensor("ckw", [TS, HD], BF, kind="ExternalInput")
    skw = nc.dram_tensor("skw", [TS, HD], BF, kind="ExternalInput")
    out_shard = nc.dram_tensor("out_shard", [TS, HID], F32,
                               kind="ExternalOutput")

    rg = [list(range(NC_))]
    stages = {}
    nc._stage_ids = stages

    def mark(name):
        stages[name] = nc.next_id()

    with tile.TileContext(nc) as tc:
        with (
            tc.tile_pool(name="dram", bufs=1, space="DRAM") as dram,
            tc.tile_pool(name="glob", bufs=1) as glob,
            tc.tile_pool(name="nrm", bufs=3) as nrm,
            tc.tile_pool(name="psP", bufs=1, space="PSUM") as psP,
        ):
            # DRAM scratch. a1 blocks are 768 rows per core:
            #   rows 0:256   q-head d  [dim, tok]
            #   rows 256:512 k-head d//2 [dim, tok]
            #   rows 512:768 v-head d//2 [tok, dim]
            a1_in = dram.tile([NC_ * 768, TS], BF)
            a1_out = dram.tile([NC_ * 768, TS], BF)
            a2_in = dram.tile([S, TS], BF)
            a2_out = dram.tile([S, TS], BF)
            x2_spill = dram.tile([TS, HID], F32)

            ident = glob.tile([128, 128], BF)
            make_identity(nc, ident[:])
            eps_t = glob.tile([128, 1], F32)
            nc.vector.memset(eps_t[:], EPS)

            def rmsnorm_rinv(src_ap, d, name):
                """rinv[p,1] = 1/sqrt(mean(src^2)+EPS) via bn_stats+Rsqrt."""
                nsub = max(1, d // 512)
                stats = nrm.tile([128, nsub, 6], F32, tag="nst", name=f"{name}_st")
                if nsub > 1:
                    view = src_ap.rearrange("p (s f) -> p s f", s=nsub)
                    for i in range(nsub):
                        nc.vector.bn_stats(out=stats[:, i, :], in_=view[:, i, :])
                else:
                    nc.vector.bn_stats(out=stats[:, 0, :], in_=src_ap)
                mv = nrm.tile([128, 2], F32, tag="nmv", name=f"{name}_mv")
                nc.vector.bn_aggr(out=mv[:], in_=stats[:])
                ms = nrm.tile([128, 1], F32, tag="nms", name=f"{name}_ms")
                nc.vector.scalar_tensor_tensor(ms[:], mv[:, 0:1], mv[:, 0:1],
                                               mv[:, 1:2], op0=ALU.mult, op1=ALU.add)
                sq = nrm.tile([128, 1], F32, tag="nsq", name=f"{name}_sq")
                nc.scalar.activation(sq[:], ms[:], AF.Sqrt, bias=eps_t[:])
                rinv = nrm.tile([128, 1], F32, tag="nrv", name=f"{name}_rv")
                nc.vector.reciprocal(rinv[:], sq[:])
                return rinv

            def c3_matmuls(psum_ap, stat_tile_idx, mov_tile_idx, i, first, last):
                """3 compensated DoubleRow matmuls for hid pair-chunk i.

                stat_tile_idx(k, hl_slice) and mov_tile_idx(k, hl_slice)
                return APs; hl_slice 0 -> hi only [.., 2chunks, n],
                'pair' -> [.., 2, n] (hi,lo of one chunk)."""
                a, b = 2 * i, 2 * i + 1
                nc.tensor.matmul(psum_ap, stat_tile_idx(a, "hi2"),
                                 mov_tile_idx(a, "hi2"),
                                 start=first, stop=False, perf_mode=DR)
                nc.tensor.matmul(psum_ap, stat_tile_idx(a, "pair"),
                                 mov_tile_idx(a, "rev"),
                                 start=False, stop=False, perf_mode=DR)
                nc.tensor.matmul(psum_ap, stat_tile_idx(b, "pair"),
                                 mov_tile_idx(b, "rev"),
                                 start=False, stop=last, perf_mode=DR)

            with tc.tile_pool(name="xpool", bufs=1) as xpool:
                x_sb = [xpool.tile([128, HID], F32, name=f"xt{t}") for t in range(2)]

                mark('S1')
                # ============ S1: in_ln + fp8 hi/lo + transpose ============
                with tc.tile_pool(name="s12", bufs=1) as s12:
                    # hT8 free layout (k, hl, tok): strides (512, 256, 1)
                    hT8 = s12.tile([128, KH, 2, TS], F8)
                    with tc.tile_pool(name="s1", bufs=2) as s1:
                        w1_in_b = s1.tile([128, HID], BF, bufs=1)
                        _bcast_row(nc, w1_in_b, w1_in, HID)
                        for t in range(2):
                            nc.sync.dma_start(x_sb[t][:],
                                              x_shard.ap()[t * 128:(t + 1) * 128, :])
                            rinv = rmsnorm_rinv(x_sb[t][:], HID, f"inln{t}")
                            h16 = s1.tile([128, HID], BF, tag="h16",
                                          name=f"h16_{t}", bufs=2)
                            nc.vector.scalar_tensor_tensor(
                                h16[:], x_sb[t][:], rinv[:], w1_in_b[:],
                                op0=ALU.mult, op1=ALU.mult)
                            for k in range(KH):
                                ptr = psP.tile([128, 128], BF, tag="tr",
                                               bufs=2, name=f"s1t{t}_{k}")
                                nc.tensor.transpose(
                                    ptr[:], h16[:, k * 128:(k + 1) * 128],
                                    ident[:])
                                ts_ = slice(t * 128, (t + 1) * 128)
                                nc.vector.tensor_copy(hT8[:, k, 0, ts_], ptr[:])
                                nc.vector.tensor_sub(hT8[:, k, 1, ts_], ptr[:],
                                                     hT8[:, k, 0, ts_])

                    mark('S2')
                    # ============ S2: QKV (C3 DR) + qk norm + rope ============
                    with tc.tile_pool(name="s2", bufs=2) as s2:
                        # [128, 2 half, 12 slot(8q+4k), TS] bf16
                        qkT = s2.tile([128, 2, 12, TS], BF, bufs=1)
                        v_sb = [s2.tile([128, NKV, HD], BF, name=f"vsb{t}",
                                        bufs=1) for t in range(2)]
                        tabs = {}
                        for nm, tab_c, tab_s in (("q", cqw, sqw), ("k", ckw, skw)):
                            for t in range(2):
                                ct = s2.tile([128, HD], BF, name=f"c{nm}{t}", bufs=1)
                                st_ = s2.tile([128, HD], BF, name=f"s{nm}{t}", bufs=1)
                                nc.sync.dma_start(
                                    ct[:], tab_c.ap()[t * 128:(t + 1) * 128, :])
                                nc.sync.dma_start(
                                    st_[:], tab_s.ap()[t * 128:(t + 1) * 128, :])
                                tabs[(nm, t)] = (ct, st_)

                        for d in range(ND):
                            wd_sb = s2.tile([128, KH, 2, 256], F8, tag="wqkv",
                                            name=f"wqkv{d}", bufs=3)
                            nc.gpsimd.dma_start(
                                wd_sb[:],
                                wqkv8.ap()[d * 128:(d + 1) * 128, :]
                                .rearrange("p (k h c) -> p k h c", k=KH, h=2))
                            for t in range(2):
                                pqkv = psP.tile([128, 256], F32, tag="mm", bufs=6,
                                                name=f"pqkv{d}_{t}")

                                def stat_f(k, kind, _t=t):
                                    if kind == "hi2":
                                        return hT8[:, k:k + 2, 0,
                                                   _t * 128:(_t + 1) * 128]
                                    return hT8[:, k, 0:2, _t * 128:(_t + 1) * 128]

                                def mov_f(k, kind, _w=wd_sb):
                                    if kind == "hi2":
                                        return _w[:, k:k + 2, 0, :]
                                    return _lohi(_w[:, k, 0:2, :], 256)

                                for i in range(KP):
                                    c3_matmuls(pqkv[:], stat_f, mov_f, i,
                                               i == 0, i == KP - 1)

                                if d < 12:  # q (0..7) or k (8..11): norm+rope
                                    nm = "q" if d < 8 else "k"
                                    slot = d
                                    rinv = rmsnorm_rinv(pqkv[:], HD,
                                                        f"{nm}n{d}_{t}")
                                    ct, st_ = tabs[(nm, t)]
                                    srcp = pqkv[:]
                                    swp = bass.AP(
                                        tensor=srcp.tensor,
                                        offset=srcp.offset + HALF,
                                        ap=[list(srcp.ap[0]), [-HALF, 2],
                                            [1, HALF]])
                                    t1 = s2.tile([128, HD], BF, tag="t1",
                                                 name=f"t1{d}_{t}", bufs=3)
                                    t2 = s2.tile([128, HD], BF, tag="t2",
                                                 name=f"t2{d}_{t}", bufs=3)
                                    nc.vector.scalar_tensor_tensor(
                                        t1[:], srcp, rinv[:], ct[:],
                                        op0=ALU.mult, op1=ALU.mult)
                                    nc.vector.scalar_tensor_tensor(
                                        t2[:].rearrange("p (a b) -> p a b", a=2),
                                        swp, rinv[:],
                                        st_[:].rearrange("p (a b) -> p a b", a=2),
                                        op0=ALU.mult, op1=ALU.mult)
                                    qr = s2.tile([128, HD], BF, tag="qr",
                                                 name=f"qr{d}_{t}", bufs=3)
                                    nc.vector.tensor_add(qr[:], t1[:], t2[:])
                                    ptr = psP.tile([128, HD], BF, tag="tr",
                                                   bufs=2, name=f"s2t{d}_{t}")
                                    for h in range(2):
                                        nc.tensor.transpose(
                                            ptr[:, h * 128:(h + 1) * 128],
                                            qr[:, h * 128:(h + 1) * 128],
                                            ident[:])
                                    nc.vector.tensor_copy(
                                        qkT[:, :, slot, t * 128:(t + 1) * 128],
                                        ptr[:].rearrange("p (a b) -> p a b", a=2))
                                else:  # v
                                    g = d - 12
                                    nc.scalar.activation(
                                        v_sb[t][:, g, :], pqkv[:], AF.Copy,
                                        scale=1.0 / WSC)

                        # ---- A2A1 assembly ----
                        a1 = a1_in[:]
                        R = TS  # row stride in elements

                        def a1_ap(dims, offset):
                            return bass.AP(tensor=a1.tensor,
                                           offset=a1.offset + offset, ap=dims)

                        # q: dest d rows [d*768 + c*128 + p]; per-c 3D DMAs
                        for c in range(2):
                            nc.sync.dma_start(
                                a1_ap([[R, 128], [768 * R, 8], [1, TS]],
                                      c * 128 * R),
                                qkT[:, c, 0:8, :])
                        # k: head g -> dests 2g, 2g+1, rows offset 256
                        for c in range(2):
                            for dup in range(2):
                                nc.sync.dma_start(
                                    a1_ap([[R, 128], [2 * 768 * R, 4], [1, TS]],
                                          (256 + c * 128 + dup * 768) * R),
                                    qkT[:, c, 8:12, :])
                        # v: rows [d*768 + 512 + t*128 + p], cols = hd
                        for t in range(2):
                            for dup in range(2):
                                nc.sync.dma_start(
                                    a1_ap([[R, 128], [2 * 768 * R, 4], [1, HD]],
                                          (512 + t * 128 + dup * 768) * R),
                                    v_sb[t][:])
                    _coll("AllToAll", ALU.bypass, [a1_in[:]], [a1_out[:]])

                mark('S3')
                # ============ S3: per-head attention (bf16) ============
                with tc.tile_pool(name="att3", bufs=1) as att3:
                    QTm = att3.tile([128, 2, 8, TS], BF)
                    KTm = att3.tile([128, 2, 8, TS], BF)
                    V = [att3.tile([128, HD + 1], BF, name=f"V{b}")
                         for b in range(S // 128)]
                    aTm = att3.tile([128, 2, 8, TS], BF)
                    masks = att3.tile([128, 8, 512], BF)
                    for i in range(8):
                        delta = 512 - 128 * i
                        mk = masks[:, i, :]
                        nc.gpsimd.memset(mk, 1.0)
                        nc.gpsimd.affine_select(
                            out=mk, in_=mk, compare_op=ALU.is_ge, fill=0.0,
                            base=delta, pattern=[[1, 512]], channel_multiplier=-1)
                        nc.gpsimd.affine_select(
                            out=mk, in_=mk, compare_op=ALU.is_ge, fill=0.0,
                            base=-delta + (WIN - 1), pattern=[[-1, 512]],
                            channel_multiplier=1)
                    a1o = a1_out[:]
                    R = TS

                    def a1o_ap(dims, offset):
                        return bass.AP(tensor=a1o.tensor,
                                       offset=a1o.offset + offset, ap=dims)

                    for c in range(2):
                        nc.sync.dma_start(
                            QTm[:, c, :, :],
                            a1o_ap([[R, 128], [768 * R, 8], [1, TS]],
                                   c * 128 * R))
                        nc.sync.dma_start(
                            KTm[:, c, :, :],
                            a1o_ap([[R, 128], [768 * R, 8], [1, TS]],
                                   (256 + c * 128) * R))
                    for b in range(S // 128):
                        s_, half = b // 2, b % 2
                        nc.sync.dma_start(
                            V[b][:, 0:HD],
                            a1o_ap([[R, 128], [1, HD]],
                                   (s_ * 768 + 512 + half * 128) * R))
                        nc.vector.memset(V[b][:, HD:HD + 1], 1.0)

                    with tc.tile_pool(name="s3", bufs=2) as s3:
                        for qb in range(4):
                            q0 = 512 * qb
                            probs = {}
                            for i in range(8):
                                kc = q0 - 512 + 128 * i
                                if kc < 0:
                                    continue
                                psc = psP.tile([128, 512], F32, tag="mm", bufs=6,
                                               name=f"psc{qb}_{i}")
                                b = kc // 128
                                for h in range(2):
                                    nc.tensor.matmul(
                                        psc[:],
                                        KTm[:, h, b // 2,
                                            (b % 2) * 128:(b % 2) * 128 + 128],
                                        QTm[:, h, 2 * qb:2 * qb + 2, :],
                                        start=(h == 0), stop=(h == 1))
                                pr = s3.tile([128, 512], BF, tag="pr",
                                             name=f"pr{qb}_{i}", bufs=10)
                                nc.scalar.activation(pr[:], psc[:], AF.Exp,
                                                     scale=1.0 / 16.0)
                                nc.vector.tensor_mul(pr[:], pr[:], masks[:, i, :])
                                probs[kc] = pr
                            for qs in range(4):
                                qa = q0 + 128 * qs
                                kcs = [kc for kc in range(qa - 512, qa + 128, 128)
                                       if kc >= 0]
                                po = psP.tile([128, HD + 1], F32, tag="mm", bufs=6,
                                              name=f"po{qb}_{qs}")
                                col = qa - q0
                                for j, kc in enumerate(kcs):
                                    nc.tensor.matmul(po[:],
                                                     probs[kc][:, col:col + 128],
                                                     V[kc // 128][:],
                                                     start=(j == 0),
                                                     stop=(j == len(kcs) - 1))
                                rec = s3.tile([128, 1], F32, tag="rec",
                                              name=f"rec{qb}_{qs}")
                                nc.vector.reciprocal(rec[:], po[:, HD:HD + 1])
                                an = s3.tile([128, HD], BF, tag="an",
                                             name=f"an{qb}_{qs}")
                                nc.vector.tensor_scalar_mul(an[:], po[:, 0:HD],
                                                            rec[:])
                                ptr = psP.tile([128, HD], BF, tag="tr", bufs=2,
                                               name=f"s3tr{qb}{qs}")
                                for h in range(2):
                                    nc.tensor.transpose(
                                        ptr[:, h * 128:(h + 1) * 128],
                                        an[:, h * 128:(h + 1) * 128], ident[:])
                                jd, jh = qa // 256, (qa // 128) % 2
                                nc.vector.tensor_copy(
                                    aTm[:, :, jd, jh * 128:jh * 128 + 128],
                                    ptr[:].rearrange("p (a b) -> p a b", a=2))
                        # A2A2: dest j gets my head's attn^T for its tokens
                        a2 = a2_in[:]
                        for c in range(2):
                            nc.sync.dma_start(
                                bass.AP(tensor=a2.tensor,
                                        offset=a2.offset + c * 128 * R,
                                        ap=[[R, 128], [256 * R, 8], [1, TS]]),
                                aTm[:, c, :, :])
                    _coll("AllToAll", ALU.bypass, [a2_in[:]], [a2_out[:]])

                mark('S4')
                # ======== S4: wo (bf16) + norms + h2 hi/lo transpose ========
                with tc.tile_pool(name="mlp", bufs=1) as mlp:
                    h2T8 = mlp.tile([128, KH, 2, TS], F8)
                    actT8 = mlp.tile([128, MT, 2, TS], F8)
                    mlp16 = [mlp.tile([128, HID], BF, name=f"mlp16_{t}")
                             for t in range(2)]
                    with tc.tile_pool(name="s4", bufs=2) as s4:
                        w1_pa_b = s4.tile([128, HID], BF, bufs=1)
                        w1_pf_b = s4.tile([128, HID], BF, bufs=1)
                        _bcast_row(nc, w1_pa_b, w1_pa, HID)
                        _bcast_row(nc, w1_pf_b, w1_pf, HID)
                        asl_v = a2_out[:].rearrange("(k p) t -> p k t", p=128)
                        asl = [s4.tile([128, 16, 128], BF, name=f"asl{t}",
                                       bufs=1) for t in range(2)]
                        ao32 = [s4.tile([128, HID], F32, name=f"ao32_{t}",
                                        bufs=1) for t in range(2)]
                        for t in range(2):
                            nc.sync.dma_start(asl[t][:],
                                              asl_v[:, :, t * 128:(t + 1) * 128])
                        for n in range(5):
                            won = s4.tile([128, 16, 512], BF, tag="won",
                                          name=f"won{n}", bufs=2)
                            nc.gpsimd.dma_start(
                                won[:],
                                wo16.ap()[:, n * 512:(n + 1) * 512]
                                .rearrange("(k p) c -> p k c", p=128))
                            for t in range(2):
                                pw = psP.tile([128, 512], F32, tag="mm", bufs=6,
                                              name=f"pw{t}_{n}")
                                for k in range(16):
                                    nc.tensor.matmul(pw[:], asl[t][:, k, :],
                                                     won[:, k, :],
                                                     start=(k == 0),
                                                     stop=(k == 15))
                                if t == 0:
                                    nc.vector.tensor_copy(
                                        ao32[t][:, n * 512:(n + 1) * 512], pw[:])
                                else:
                                    nc.scalar.activation(
                                        ao32[t][:, n * 512:(n + 1) * 512], pw[:],
                                        AF.Copy)
                        for t in range(2):
                            rinv_a = rmsnorm_rinv(ao32[t][:], HID, f"pan{t}")
                            x2 = s4.tile([128, HID], F32, tag="x2",
                                         name=f"x2_{t}", bufs=2)
                            nc.vector.scalar_tensor_tensor(
                                x2[:], ao32[t][:], rinv_a[:], w1_pa_b[:],
                                op0=ALU.mult, op1=ALU.mult)
                            nc.vector.tensor_add(x2[:], x2[:], x_sb[t][:])
                            nc.sync.dma_start(
                                x2_spill[t * 128:(t + 1) * 128, :], x2[:])
                            rinv_f = rmsnorm_rinv(x2[:], HID, f"pff{t}")
                            h216 = s4.tile([128, HID], BF, tag="h216",
                                           name=f"h216_{t}", bufs=2)
                            nc.vector.scalar_tensor_tensor(
                                h216[:], x2[:], rinv_f[:], w1_pf_b[:],
                                op0=ALU.mult, op1=ALU.mult)
                            for k in range(KH):
                                ptr = psP.tile([128, 128], BF, tag="tr",
                                               bufs=2, name=f"s4t{t}{k}")
                                nc.tensor.transpose(
                                    ptr[:], h216[:, k * 128:(k + 1) * 128],
                                    ident[:])
                                ts_ = slice(t * 128, (t + 1) * 128)
                                nc.vector.tensor_copy(h2T8[:, k, 0, ts_], ptr[:])
                                nc.vector.tensor_sub(h2T8[:, k, 1, ts_], ptr[:],
                                                     h2T8[:, k, 0, ts_])

                    mark('S5')
                    # ============ S5: data-parallel MLP (C3 DR) ============
                    with tc.tile_pool(name="s5", bufs=2) as s5:
                        def h2_mov(k, kind):
                            if kind == "hi2":
                                return h2T8[:, k:k + 2, 0, :]
                            return _lohi(h2T8[:, k, 0:2, :], TS)

                        for m in range(MT):
                            wgm = s5.tile([128, KH, 2, 128], F8, tag="wgm",
                                          name=f"wgm{m}", bufs=4)
                            wum = s5.tile([128, KH, 2, 128], F8, tag="wum",
                                          name=f"wum{m}", bufs=4)
                            nc.gpsimd.dma_start(
                                wgm[:], wg8.ap()[m * 128:(m + 1) * 128, :]
                                .rearrange("p (k h c) -> p k h c", k=KH, h=2))
                            nc.gpsimd.dma_start(
                                wum[:], wu8.ap()[m * 128:(m + 1) * 128, :]
                                .rearrange("p (k h c) -> p k h c", k=KH, h=2))
                            pg = psP.tile([128, TS], F32, tag="mm", bufs=6,
                                          name=f"pg{m}")
                            pu = psP.tile([128, TS], F32, tag="mm", bufs=6,
                                          name=f"pu{m}")

                            def wg_stat(k, kind, _w=wgm):
                                if kind == "hi2":
                                    return _w[:, k:k + 2, 0, :]
                                return _w[:, k, 0:2, :]

                            def wu_stat(k, kind, _w=wum):
                                if kind == "hi2":
                                    return _w[:, k:k + 2, 0, :]
                                return _w[:, k, 0:2, :]

                            for i in range(KP):
                                c3_matmuls(pg[:], wg_stat, h2_mov, i,
                                           i == 0, i == KP - 1)
                                c3_matmuls(pu[:], wu_stat, h2_mov, i,
                                           i == 0, i == KP - 1)
                            gsc = s5.tile([128, TS], F32, tag="gsc",
                                          name=f"gsc{m}", bufs=3)
                            nc.scalar.activation(gsc[:], pg[:],
                                                 AF.Gelu_apprx_tanh,
                                                 scale=1.0 / WSC)
                            gu32 = s5.tile([128, TS], F32, tag="gu32",
                                           name=f"gu32_{m}", bufs=3)
                            nc.vector.scalar_tensor_tensor(
                                gu32[:], pu[:], 1.0 / WSC, gsc[:],
                                op0=ALU.mult, op1=ALU.mult)
                            nc.vector.tensor_copy(actT8[:, m, 0, :], gu32[:])
                            nc.vector.tensor_sub(actT8[:, m, 1, :], gu32[:],
                                                 actT8[:, m, 0, :])

                        def act_mov(j, kind):
                            if kind == "hi2":
                                return actT8[:, j:j + 2, 0, :]
                            return _lohi(actT8[:, j, 0:2, :], TS)

                        for n in range(NT):
                            wdn = s5.tile([128, MT, 2, 128], F8, tag="wdn",
                                          name=f"wdn{n}", bufs=2)
                            nc.gpsimd.dma_start(
                                wdn[:], wd8.ap()[n * 128:(n + 1) * 128, :]
                                .rearrange("p (j h c) -> p j h c", j=MT, h=2))
                            pd = psP.tile([128, TS], F32, tag="mm", bufs=6,
                                          name=f"pd{n}")

                            def wd_stat(j, kind, _w=wdn):
                                if kind == "hi2":
                                    return _w[:, j:j + 2, 0, :]
                                return _w[:, j, 0:2, :]

                            for j in range(JP):
                                c3_matmuls(pd[:], wd_stat, act_mov, j,
                                           j == 0, j == JP - 1)
                            md = s5.tile([128, TS], BF, tag="md",
                                         name=f"md{n}", bufs=3)
                            nc.scalar.activation(md[:], pd[:], AF.Copy,
                                                 scale=1.0 / WSC)
                            for t in range(2):
                                ptr = psP.tile([128, 128], BF, tag="tr", bufs=2,
                                               name=f"s5t{n}_{t}")
                                nc.tensor.transpose(
                                    ptr[:], md[:, t * 128:(t + 1) * 128],
                                    ident[:])
                                nc.vector.tensor_copy(
                                    mlp16[t][:, n * 128:(n + 1) * 128], ptr[:])

                    mark('S6')
                    # ============ S6: post_ff norm + residual ============
                    with tc.tile_pool(name="s6", bufs=2) as s6:
                        w1_po_b = s6.tile([128, HID], F32, bufs=1)
                        _bcast_row(nc, w1_po_b, w1_po, HID)
                        for t in range(2):
                            x2l = s6.tile([128, HID], F32, tag="x2l",
                                          name=f"x2l{t}", bufs=2)
                            nc.sync.dma_start(
                                x2l[:], x2_spill[t * 128:(t + 1) * 128, :])
                            rinv_o = rmsnorm_rinv(mlp16[t][:], HID, f"pon{t}")
                            o32 = s6.tile([128, HID], F32, tag="o32",
                                          name=f"o32_{t}", bufs=2)
                            nc.vector.scalar_tensor_tensor(
                                o32[:], mlp16[t][:], rinv_o[:], w1_po_b[:],
                                op0=ALU.mult, op1=ALU.mult)
                            nc.vector.tensor_add(o32[:], o32[:], x2l[:])
                            nc.sync.dma_start(
                                out_shard.ap()[t * 128:(t + 1) * 128, :], o32[:])

    nc.compile()
    return nc


_NC_CACHE = None


def _get_nc():
    global _NC_CACHE
    if _NC_CACHE is None:
        _NC_CACHE = build_nc()
    return _NC_CACHE


def _hi_lo(w):
    f8 = ml_dtypes.float8_e4m3
    hi = w.astype(f8)
    lo = (w - hi.astype(np.float32)).astype(f8)
    return hi, lo


def _pack_pairs(w, kchunks, ntiles):
    """w [kchunks*128, ntiles*128*cw] f32 -> [ntiles*128, kchunks*2*cw] f8
    with row n*128+p, col k*(2cw)+hl*cw+c = hl-part of w[k*128+p, n*cw+c]."""
    K, N = w.shape
    cw = N // ntiles
    hi, lo = _hi_lo(w)
    st = np.stack([hi, lo], 0).reshape(2, kchunks, 128, ntiles, cw)
    return np.ascontiguousarray(
        st.transpose(3, 2, 1, 0, 4).reshape(ntiles * 128, kchunks * 2 * cw))


def make_in_maps(hidden_states, position_ids, wq, wk, wv, wo, q_ln_w, k_ln_w,
                 in_ln_w, post_attn_ln_w, pre_ff_ln_w, post_ff_ln_w,
                 w_gate, w_up, w_down):
    bf16 = ml_dtypes.bfloat16
    f32 = np.float32
    x = np.asarray(hidden_states, f32).reshape(S, HID)
    pos = np.asarray(position_ids).reshape(S).astype(np.float64)

    inv_freq = 1.0 / (BASE ** (np.arange(0, HD, 2, dtype=np.float64) / HD))
    freqs = pos[:, None] * inv_freq[None, :]
    emb = np.concatenate([freqs, freqs], axis=1)
    cos = np.cos(emb).astype(f32)
    sin = np.sin(emb).astype(f32)
    w1q = 1.0 + np.asarray(q_ln_w, f32)
    w1k = 1.0 + np.asarray(k_ln_w, f32)

    def rope_tabs(w1):
        w1sw = np.concatenate([w1[HALF:], w1[:HALF]])
        sgn = np.concatenate([-np.ones(HALF, f32), np.ones(HALF, f32)])
        return ((cos * w1[None, :]).astype(bf16),
                (sin * (w1sw * sgn)[None, :]).astype(bf16))

    cqw_np, sqw_np = rope_tabs(w1q)
    ckw_np, skw_np = rope_tabs(w1k)

    wqkv = np.concatenate([np.asarray(wq, f32), np.asarray(wk, f32),
                           np.asarray(wv, f32)], axis=1) * WSC
    wqkv8_np = _pack_pairs(wqkv, KH, ND)          # chunk width 256
    wg8_np = _pack_pairs(np.asarray(w_gate, f32) * WSC, KH, MT)
    wu8_np = _pack_pairs(np.asarray(w_up, f32) * WSC, KH, MT)
    wd8_np = _pack_pairs(np.asarray(w_down, f32) * WSC, MT, NT)

    common = {
        "wqkv8": wqkv8_np,
        "wo16": np.asarray(wo, f32).astype(bf16),
        "wg8": wg8_np, "wu8": wu8_np, "wd8": wd8_np,
        "w1_in": (1.0 + np.asarray(in_ln_w, f32)).astype(bf16),
        "w1_pa": (1.0 + np.asarray(post_attn_ln_w, f32)).astype(bf16),
        "w1_pf": (1.0 + np.asarray(pre_ff_ln_w, f32)).astype(bf16),
        "w1_po": 1.0 + np.asarray(post_ff_ln_w, f32),
    }
    in_maps = []
    for c in range(NC_):
        sl = slice(c * TS, (c + 1) * TS)
        in_maps.append({
            "x_shard": np.ascontiguousarray(x[sl]),
            "cqw": np.ascontiguousarray(cqw_np[sl]),
            "sqw": np.ascontiguousarray(sqw_np[sl]),
            "ckw": np.ascontiguousarray(ckw_np[sl]),
            "skw": np.ascontiguousarray(skw_np[sl]),
            **common,
        })
    return in_maps


def kernel(**inputs):
    in_maps = make_in_maps(**inputs)
    nc = _get_nc()
    res = run_bass_kernel_spmd(nc, in_maps, core_ids=list(range(NC_)))
    out = np.concatenate([res.results[c]["out_shard"] for c in range(NC_)],
                         axis=0)
    return out.reshape(1, S, HID).astype(np.float32)


# revision 34
# speedup vs baseline: 1.2698x; 1.1169x over previous
"""Gemma3 decoder layer on 8 Trainium2 NeuronCores (Bass/Tile), v2.

Sharding / dataflow (per core c, tokens [256c, 256c+256)):
  S1  in_ln locally; quantize h to fp8 hi+lo pair; transpose -> hT8.
  S2  QKV for ALL heads of MY tokens via compensated-fp8 DoubleRow matmuls
      (3 DR matmuls per 256-wide hid pair-chunk: hi*hi both chunks, plus the
      two hi*lo cross terms; ~bf16 accuracy at 0.75x bf16 PE cost under the
      DoubleRow perf model). Per-head q/k rmsnorm + rope applied locally in
      [tok, dim] layout, then transposed and shipped per-head via AllToAll.
  S3  per-head sliding-window attention in bf16 (accuracy-critical);
      attn^T shipped back to token owners via a second AllToAll.
  S4  wo in bf16 (streamed weight), post_attn norm + residual + pre_ff norm;
      h2 quantized to fp8 hi+lo and transposed -> h2T8.
  S5  MLP fully data-parallel over tokens: every core streams the FULL
      gate/up/down weights as fp8 hi+lo pairs (same bytes as bf16) and runs
      compensated DoubleRow matmuls. No AllGather / ReduceScatter at all.
  S6  post_ff norm + residual locally.
Weights are pre-scaled by 256 on the host; the scale is absorbed by the
rmsnorms (scale-invariant) or folded into activation scales (gelu, copies).
"""
import sys

if "/opt/trn_rl_repo" not in sys.path:
    sys.path.insert(0, "/opt/trn_rl_repo")

import numpy as np
import ml_dtypes

import concourse.bass as bass
import concourse.mybir as mybir
import concourse.tile as tile
from concourse import bacc
from concourse.bass_utils import run_bass_kernel_spmd
from concourse.masks import make_identity

dt = mybir.dt
AF = mybir.ActivationFunctionType
ALU = mybir.AluOpType
BF = dt.bfloat16
F32 = dt.float32
F8 = dt.float8e4
DR = mybir.MatmulPerfMode.DoubleRow

HID, NH, NKV, HD, INTER = 2560, 8, 4, 256, 10240
WIN, EPS, BASE = 512, 1e-6, 10000.0
S = 2048
NC_ = 8
TS = S // NC_              # 256 tokens per core
KH = HID // 128            # 20 hid 128-chunks
KP = KH // 2               # 10 hid pair-chunks (256 contraction per DR)
ND = (NH + NKV + NKV)      # 16 qkv dim chunks of 256 (8 q | 4 k | 4 v)
MT = INTER // 128          # 80 inter tiles
JP = MT // 2               # 40 inter pair-chunks
NT = HID // 128            # 20 hid output tiles
HALF = HD // 2
WSC = 256.0                # host-side weight scale (power of two)
WGB = 6                    # gate/up weight-stream ring depth (tiles)
PF2 = 2                    # extra prefetch tiles allocated at S4
WOB = 2                    # wo weight-stream ring depth


def _bcast_row(nc, sbuf_tile, dram_t, width):
    a = dram_t.ap()
    nc.sync.dma_start(sbuf_tile[:], bass.AP(
        tensor=a.tensor, offset=a.offset, ap=[[0, 128], [1, width]]))


def _lohi(t_ap, n):
    """Given ap [...,2,n] positioned at (hi,lo), return [128, 2, n] iterating
    (lo, hi)."""
    a = t_ap
    return bass.AP(tensor=a.tensor, offset=a.offset + n,
                   ap=[list(a.ap[0]), [-n, 2], [1, n]])


def build_nc(sim=False, taps=False):
    nc = bacc.Bacc("TRN2", target_bir_lowering=False, debug=False,
                   enable_asserts=True, num_devices=1 if sim else NC_)

    def _coll(kind, op, ins, outs):
        if not sim:
            nc.gpsimd.collective_compute(kind, op, replica_groups=rg,
                                         ins=ins, outs=outs)
            return
        i_ap, o_ap = ins[0], outs[0]
        if kind == "AllToAll":
            nc.sync.dma_start(o_ap, i_ap)

    x_shard = nc.dram_tensor("x_shard", [TS, HID], F32, kind="ExternalInput")
    wqkv8 = nc.dram_tensor("wqkv8", [ND * 128, KH * 2 * 256], F8,
                           kind="ExternalInput")
    wo16 = nc.dram_tensor("wo16", [NH * HD, HID], BF, kind="ExternalInput")
    wgu8 = nc.dram_tensor("wgu8", [MT * 128, 2 * KH * 2 * 128], F8,
                          kind="ExternalInput")
    wd8 = nc.dram_tensor("wd8", [NT * 128, MT * 128], F8,
                         kind="ExternalInput")
    w1_in = nc.dram_tensor("w1_in", [HID], BF, kind="ExternalInput")
    w1_pa = nc.dram_tensor("w1_pa", [HID], BF, kind="ExternalInput")
    w1_pf = nc.dram_tensor("w1_pf", [HID], BF, kind="ExternalInput")
    w1_po = nc.dram_tensor("w1_po", [HID], F32, kind="ExternalInput")
    cqw = nc.dram_tensor("cqw", [TS, HD], BF, kind="ExternalInput")
    sqw = nc.dram_tensor("sqw", [TS, HD], BF, kind="ExternalInput")
    ckw = nc.dram_tensor("ckw", [TS, HD], BF, kind="ExternalInput")
    skw = nc.dram_tensor("skw", [TS, HD], BF, kind="ExternalInput")
    out_shard = nc.dram_tensor("out_shard", [TS, HID], F32,
                               kind="ExternalOutput")
    if taps:
        dbg_x2 = nc.dram_tensor("dbg_x2", [TS, HID], F32, kind="ExternalOutput")
        dbg_h2 = nc.dram_tensor("dbg_h2", [TS, HID], BF, kind="ExternalOutput")
        dbg_gu = nc.dram_tensor("dbg_gu", [128, TS], F32, kind="ExternalOutput")
        dbg_mlp = nc.dram_tensor("dbg_mlp", [TS, HID], BF,
                                 kind="ExternalOutput")
        dbg_qt = nc.dram_tensor("dbg_qt", [128, 2 * 8 * TS], BF,
                                kind="ExternalOutput")
        dbg_kt = nc.dram_tensor("dbg_kt", [128, 2 * 8 * TS], BF,
                                kind="ExternalOutput")
        dbg_at = nc.dram_tensor("dbg_at", [128, 2 * 8 * TS], BF,
                                kind="ExternalOutput")
        dbg_ao = nc.dram_tensor("dbg_ao", [TS, HID], F32, kind="ExternalOutput")
        dbg_hT = nc.dram_tensor("dbg_hT", [128, KH * 2 * TS], F8,
                                kind="ExternalOutput")

    rg = [list(range(NC_))]
    stages = {}
    nc._stage_ids = stages

    def mark(name):
        stages[name] = nc.next_id()

    with tile.TileContext(nc) as tc:
        with (
            tc.tile_pool(name="dram", bufs=1, space="DRAM") as dram,
            tc.tile_pool(name="glob", bufs=1) as glob,
            tc.tile_pool(name="nrm", bufs=3) as nrm,
            tc.tile_pool(name="psP", bufs=1, space="PSUM") as psP,
        ):
            # DRAM scratch. a1 blocks are 768 rows per core:
            #   rows 0:256   q-head d  [dim, tok]
            #   rows 256:512 k-head d//2 [dim, tok]
            #   rows 512:768 v-head d//2 [tok, dim]
            a1_in = dram.tile([NC_ * 768, TS], BF)
            a1_out = dram.tile([NC_ * 768, TS], BF)
            a2_in = dram.tile([S, TS], BF)
            a2_out = dram.tile([S, TS], BF)
            x2_spill = dram.tile([TS, HID], F32)

            ident = glob.tile([128, 128], BF)
            make_identity(nc, ident[:])
            eps_t = glob.tile([128, 1], F32)
            nc.vector.memset(eps_t[:], EPS)

            def rinv_from_stats(stats_ap, name):
                mv = nrm.tile([128, 2], F32, tag="nmv", name=f"{name}_mv")
                nc.vector.bn_aggr(out=mv[:], in_=stats_ap)
                ms = nrm.tile([128, 1], F32, tag="nms", name=f"{name}_ms")
                nc.vector.scalar_tensor_tensor(ms[:], mv[:, 0:1], mv[:, 0:1],
                                               mv[:, 1:2], op0=ALU.mult, op1=ALU.add)
                sq = nrm.tile([128, 1], F32, tag="nsq", name=f"{name}_sq")
                nc.scalar.activation(sq[:], ms[:], AF.Sqrt, bias=eps_t[:])
                rinv = nrm.tile([128, 1], F32, tag="nrv", name=f"{name}_rv")
                nc.vector.reciprocal(rinv[:], sq[:])
                return rinv

            def rmsnorm_rinv(src_ap, d, name):
                """rinv[p,1] = 1/sqrt(mean(src^2)+EPS) via bn_stats+Sqrt."""
                nsub = max(1, d // 512)
                stats = nrm.tile([128, nsub, 6], F32, tag="nst", name=f"{name}_st")
                if nsub > 1:
                    view = src_ap.rearrange("p (s f) -> p s f", s=nsub)
                    for i in range(nsub):
                        nc.vector.bn_stats(out=stats[:, i, :], in_=view[:, i, :])
                else:
                    nc.vector.bn_stats(out=stats[:, 0, :], in_=src_ap)
                return rinv_from_stats(stats[:], name)

            def c3_matmuls(psum_ap, stat_tile_idx, mov_tile_idx, i, first, last):
                """3 compensated DoubleRow matmuls for hid pair-chunk i.

                stat_tile_idx(k, hl_slice) and mov_tile_idx(k, hl_slice)
                return APs; hl_slice 0 -> hi only [.., 2chunks, n],
                'pair' -> [.., 2, n] (hi,lo of one chunk)."""
                a, b = 2 * i, 2 * i + 1
                nc.tensor.matmul(psum_ap, stat_tile_idx(a, "hi2"),
                                 mov_tile_idx(a, "hi2"),
                                 start=first, stop=False, perf_mode=DR)
                nc.tensor.matmul(psum_ap, stat_tile_idx(a, "pair"),
                                 mov_tile_idx(a, "rev"),
                                 start=False, stop=False, perf_mode=DR)
                nc.tensor.matmul(psum_ap, stat_tile_idx(b, "pair"),
                                 mov_tile_idx(b, "rev"),
                                 start=False, stop=last, perf_mode=DR)

            with tc.tile_pool(name="wst", bufs=1) as wst:
                mark('S1')
                # ============ S1: in_ln + fp8 hi/lo + transpose ============
                with tc.tile_pool(name="s12", bufs=1) as s12:
                    # hT8 free layout (k, hl, tok): strides (512, 256, 1)
                    hT8 = s12.tile([128, KH, 2, TS], F8)
                    with tc.tile_pool(name="s1", bufs=2) as s1:
                        w1_in_b = s1.tile([128, HID], BF, bufs=1)
                        _bcast_row(nc, w1_in_b, w1_in, HID)
                        x_sb, rinvs, h16s = [], [], []
                        for t in range(2):
                            xt = s1.tile([128, HID], F32, tag="xsb",
                                         name=f"xt{t}", bufs=2)
                            nc.sync.dma_start(xt[:],
                                              x_shard.ap()[t * 128:(t + 1) * 128, :])
                            x_sb.append(xt)
                        for t in range(2):
                            rinvs.append(rmsnorm_rinv(x_sb[t][:], HID, f"inln{t}"))
                        for t in range(2):
                            h16 = s1.tile([128, HID], BF, tag="h16",
                                          name=f"h16_{t}", bufs=2)
                            nc.vector.scalar_tensor_tensor(
                                h16[:], x_sb[t][:], rinvs[t][:], w1_in_b[:],
                                op0=ALU.mult, op1=ALU.mult)
                            h16s.append(h16)
                        for k in range(KH):
                            for t in range(2):
                                ptr = psP.tile([128, 128], BF, tag="tr",
                                               bufs=2, name=f"s1t{t}_{k}")
                                nc.tensor.transpose(
                                    ptr[:], h16s[t][:, k * 128:(k + 1) * 128],
                                    ident[:])
                                ts_ = slice(t * 128, (t + 1) * 128)
                                nc.scalar.activation(hT8[:, k, 0, ts_], ptr[:],
                                                     AF.Copy)
                                nc.vector.tensor_sub(hT8[:, k, 1, ts_], ptr[:],
                                                     hT8[:, k, 0, ts_])

                    if taps:
                        nc.sync.dma_start(dbg_hT.ap(), hT8[:])
                    mark('S2')
                    # ============ S2: QKV (C3 DR) + qk norm + rope ============
                    with tc.tile_pool(name="s2", bufs=2) as s2:
                        # [128, 2 half, 12 slot(8q+4k), TS] bf16
                        qkT = s2.tile([128, 2, 12, TS], BF, bufs=1)
                        v_sb = [s2.tile([128, NKV, HD], BF, name=f"vsb{t}",
                                        bufs=1) for t in range(2)]
                        tabs = {}
                        for nm, tab_c, tab_s in (("q", cqw, sqw), ("k", ckw, skw)):
                            for t in range(2):
                                ct = s2.tile([128, HD], BF, name=f"c{nm}{t}", bufs=1)
                                st_ = s2.tile([128, HD], BF, name=f"s{nm}{t}", bufs=1)
                                nc.sync.dma_start(
                                    ct[:], tab_c.ap()[t * 128:(t + 1) * 128, :])
                                nc.sync.dma_start(
                                    st_[:], tab_s.ap()[t * 128:(t + 1) * 128, :])
                                tabs[(nm, t)] = (ct, st_)

                        for d in range(ND):
                            wd_sb = s2.tile([128, KH, 2, 256], F8, tag="wqkv",
                                            name=f"wqkv{d}", bufs=8)
                            nc.gpsimd.dma_start(
                                wd_sb[:],
                                wqkv8.ap()[d * 128:(d + 1) * 128, :]
                                .rearrange("p (k h c) -> p k h c", k=KH, h=2))
                            for t in range(2):
                                pqkv = psP.tile([128, 256], F32, tag="mm", bufs=6,
                                                name=f"pqkv{d}_{t}")

                                def stat_f(k, kind, _t=t):
                                    if kind == "hi2":
                                        return hT8[:, k:k + 2, 0,
                                                   _t * 128:(_t + 1) * 128]
                                    return hT8[:, k, 0:2, _t * 128:(_t + 1) * 128]

                                def mov_f(k, kind, _w=wd_sb):
                                    if kind == "hi2":
                                        return _w[:, k:k + 2, 0, :]
                                    return _lohi(_w[:, k, 0:2, :], 256)

                                for i in range(KP):
                                    c3_matmuls(pqkv[:], stat_f, mov_f, i,
                                               i == 0, i == KP - 1)

                                if d < 12:  # q (0..7) or k (8..11): norm+rope
                                    nm = "q" if d < 8 else "k"
                                    slot = d
                                    rinv = rmsnorm_rinv(pqkv[:], HD,
                                                        f"{nm}n{d}_{t}")
                                    ct, st_ = tabs[(nm, t)]
                                    srcp = pqkv[:]
                                    swp = bass.AP(
                                        tensor=srcp.tensor,
                                        offset=srcp.offset + HALF,
                                        ap=[list(srcp.ap[0]), [-HALF, 2],
                                            [1, HALF]])
                                    t1 = s2.tile([128, HD], BF, tag="t1",
                                                 name=f"t1{d}_{t}", bufs=3)
                                    t2 = s2.tile([128, HD], BF, tag="t2",
                                                 name=f"t2{d}_{t}", bufs=3)
                                    nc.vector.scalar_tensor_tensor(
                                        t1[:], srcp, rinv[:], ct[:],
                                        op0=ALU.mult, op1=ALU.mult)
                                    nc.vector.scalar_tensor_tensor(
                                        t2[:].rearrange("p (a b) -> p a b", a=2),
                                        swp, rinv[:],
                                        st_[:].rearrange("p (a b) -> p a b", a=2),
                                        op0=ALU.mult, op1=ALU.mult)
                                    qr = s2.tile([128, HD], BF, tag="qr",
                                                 name=f"qr{d}_{t}", bufs=3)
                                    nc.vector.tensor_add(qr[:], t1[:], t2[:])
                                    ptr = psP.tile([128, HD], BF, tag="tr",
                                                   bufs=2, name=f"s2t{d}_{t}")
                                    for h in range(2):
                                        nc.tensor.transpose(
                                            ptr[:, h * 128:(h + 1) * 128],
                                            qr[:, h * 128:(h + 1) * 128],
                                            ident[:])
                                    nc.vector.tensor_copy(
                                        qkT[:, :, slot, t * 128:(t + 1) * 128],
                                        ptr[:].rearrange("p (a b) -> p a b", a=2))
                                else:  # v
                                    g = d - 12
                                    nc.scalar.activation(
                                        v_sb[t][:, g, :], pqkv[:], AF.Copy,
                                        scale=1.0 / WSC)

                        # ---- A2A1 assembly ----
                        a1 = a1_in[:]
                        R = TS  # row stride in elements

                        def a1_ap(dims, offset):
                            return bass.AP(tensor=a1.tensor,
                                           offset=a1.offset + offset, ap=dims)

                        # q: dest d rows [d*768 + c*128 + p]; per-c 3D DMAs
                        for c in range(2):
                            nc.sync.dma_start(
                                a1_ap([[R, 128], [768 * R, 8], [1, TS]],
                                      c * 128 * R),
                                qkT[:, c, 0:8, :])
                        # k: head g -> dests 2g, 2g+1, rows offset 256
                        for c in range(2):
                            for dup in range(2):
                                nc.sync.dma_start(
                                    a1_ap([[R, 128], [2 * 768 * R, 4], [1, TS]],
                                          (256 + c * 128 + dup * 768) * R),
                                    qkT[:, c, 8:12, :])
                        # v: rows [d*768 + 512 + t*128 + p], cols = hd
                        for t in range(2):
                            for dup in range(2):
                                nc.sync.dma_start(
                                    a1_ap([[R, 128], [2 * 768 * R, 4], [1, HD]],
                                          (512 + t * 128 + dup * 768) * R),
                                    v_sb[t][:])
                    # Pre-fill the MLP/wo weight-stream rings now: these DMAs
                    # have no dependencies, and emitting them before the
                    # collectives keeps the gpsimd queue free to prefetch
                    # while attention runs. (Count must be <= ring bufs so
                    # none of them waits on a slot recycle.)
                    def load_wgu(m):
                        w = wst.tile([128, 2, KH, 2, 128], F8, tag="wgu",
                                     name=f"wgu{m}", bufs=WGB)
                        nc.gpsimd.dma_start(
                            w[:], wgu8.ap()[m * 128:(m + 1) * 128, :]
                            .rearrange("p (g k h c) -> p g k h c", g=2, k=KH,
                                       h=2))
                        return w

                    def load_won(n):
                        w = wst.tile([128, 16, 256], BF, tag="won",
                                     name=f"won{n}", bufs=WOB)
                        nc.gpsimd.dma_start(
                            w[:], wo16.ap()[:, n * 256:(n + 1) * 256]
                            .rearrange("(k p) c -> p k c", p=128))
                        return w

                    w1_po_b = wst.tile([128, HID], F32, bufs=1)
                    _bcast_row(nc, w1_po_b, w1_po, HID)
                    wgu_pre = [load_wgu(m) for m in range(WGB)]
                    won_pre = [load_won(n) for n in range(WOB)]
                    _coll("AllToAll", ALU.bypass, [a1_in[:]], [a1_out[:]])

                mark('S3')
                # ============ S3: per-head attention (bf16) ============
                with tc.tile_pool(name="att3", bufs=1) as att3:
                    QTm = att3.tile([128, 2, 8, TS], BF)
                    KTm = att3.tile([128, 2, 8, TS], BF)
                    V = [att3.tile([128, HD + 1], BF, name=f"V{b}")
                         for b in range(S // 128)]
                    aTm = att3.tile([128, 2, 8, TS], BF)
                    masks = att3.tile([128, 8, 512], BF)
                    for i in range(8):
                        delta = 512 - 128 * i
                        mk = masks[:, i, :]
                        nc.gpsimd.memset(mk, 1.0)
                        nc.gpsimd.affine_select(
                            out=mk, in_=mk, compare_op=ALU.is_ge, fill=0.0,
                            base=delta, pattern=[[1, 512]], channel_multiplier=-1)
                        nc.gpsimd.affine_select(
                            out=mk, in_=mk, compare_op=ALU.is_ge, fill=0.0,
                            base=-delta + (WIN - 1), pattern=[[-1, 512]],
                            channel_multiplier=1)
                    a1o = a1_out[:]
                    R = TS

                    def a1o_ap(dims, offset):
                        return bass.AP(tensor=a1o.tensor,
                                       offset=a1o.offset + offset, ap=dims)

                    for c in range(2):
                        nc.sync.dma_start(
                            QTm[:, c, :, :],
                            a1o_ap([[R, 128], [768 * R, 8], [1, TS]],
                                   c * 128 * R))
                        nc.sync.dma_start(
                            KTm[:, c, :, :],
                            a1o_ap([[R, 128], [768 * R, 8], [1, TS]],
                                   (256 + c * 128) * R))
                    for b in range(S // 128):
                        s_, half = b // 2, b % 2
                        nc.sync.dma_start(
                            V[b][:, 0:HD],
                            a1o_ap([[R, 128], [1, HD]],
                                   (s_ * 768 + 512 + half * 128) * R))
                        nc.vector.memset(V[b][:, HD:HD + 1], 1.0)

                    if taps:
                        nc.sync.dma_start(dbg_qt.ap(), QTm[:])
                        nc.sync.dma_start(dbg_kt.ap(), KTm[:])
                    with tc.tile_pool(name="s3", bufs=2) as s3:
                        for qb in range(4):
                            q0 = 512 * qb
                            probs = {}
                            for i in range(8):
                                kc = q0 - 512 + 128 * i
                                if kc < 0:
                                    continue
                                psc = psP.tile([128, 512], F32, tag="mm", bufs=6,
                                               name=f"psc{qb}_{i}")
                                b = kc // 128
                                for h in range(2):
                                    nc.tensor.matmul(
                                        psc[:],
                                        KTm[:, h, b // 2,
                                            (b % 2) * 128:(b % 2) * 128 + 128],
                                        QTm[:, h, 2 * qb:2 * qb + 2, :],
                                        start=(h == 0), stop=(h == 1))
                                pr = s3.tile([128, 512], BF, tag="pr",
                                             name=f"pr{qb}_{i}", bufs=10)
                                nc.scalar.activation(pr[:], psc[:], AF.Exp,
                                                     scale=1.0 / 16.0)
                                nc.vector.tensor_mul(pr[:], pr[:], masks[:, i, :])
                                probs[kc] = pr
                            for qs in range(4):
                                qa = q0 + 128 * qs
                                kcs = [kc for kc in range(qa - 512, qa + 128, 128)
                                       if kc >= 0]
                                po = psP.tile([128, HD + 1], F32, tag="mm", bufs=6,
                                              name=f"po{qb}_{qs}")
                                col = qa - q0
                                for j, kc in enumerate(kcs):
                                    nc.tensor.matmul(po[:],
                                                     probs[kc][:, col:col + 128],
                                                     V[kc // 128][:],
                                                     start=(j == 0),
                                                     stop=(j == len(kcs) - 1))
                                rec = s3.tile([128, 1], F32, tag="rec",
                                              name=f"rec{qb}_{qs}")
                                nc.vector.reciprocal(rec[:], po[:, HD:HD + 1])
                                an = s3.tile([128, HD], BF, tag="an",
                                             name=f"an{qb}_{qs}")
                                nc.vector.tensor_scalar_mul(an[:], po[:, 0:HD],
                                                            rec[:])
                                ptr = psP.tile([128, HD], BF, tag="tr", bufs=2,
                                               name=f"s3tr{qb}{qs}")
                                for h in range(2):
                                    nc.tensor.transpose(
                                        ptr[:, h * 128:(h + 1) * 128],
                                        an[:, h * 128:(h + 1) * 128], ident[:])
                                jd, jh = qa // 256, (qa // 128) % 2
                                nc.vector.tensor_copy(
                                    aTm[:, :, jd, jh * 128:jh * 128 + 128],
                                    ptr[:].rearrange("p (a b) -> p a b", a=2))
                        if taps:
                            nc.sync.dma_start(dbg_at.ap(), aTm[:])
                        # A2A2: dest j gets my head's attn^T for its tokens
                        a2 = a2_in[:]
                        for c in range(2):
                            nc.sync.dma_start(
                                bass.AP(tensor=a2.tensor,
                                        offset=a2.offset + c * 128 * R,
                                        ap=[[R, 128], [256 * R, 8], [1, TS]]),
                                aTm[:, c, :, :])
                    _coll("AllToAll", ALU.bypass, [a2_in[:]], [a2_out[:]])

                mark('S4')
                # ======== S4: wo (bf16) + norms + h2 hi/lo transpose ========
                with tc.tile_pool(name="mlp", bufs=1) as mlp:
                    h2T8 = mlp.tile([128, KH, 2, TS], F8)
                    mlp16 = [mlp.tile([128, HID], BF, name=f"mlp16_{t}")
                             for t in range(2)]
                    # second prefetch wave: fresh slots allocated only now so
                    # they do not count against S2/S3 SBUF; their loads land
                    # in the gpsimd stream after the A2A2 dispatch.
                    pf2 = []
                    for m in range(WGB, WGB + PF2):
                        w = mlp.tile([128, 2, KH, 2, 128], F8, name=f"wgu{m}")
                        nc.gpsimd.dma_start(
                            w[:], wgu8.ap()[m * 128:(m + 1) * 128, :]
                            .rearrange("p (g k h c) -> p g k h c", g=2, k=KH,
                                       h=2))
                        pf2.append(w)
                    with tc.tile_pool(name="s4", bufs=2) as s4:
                        w1_pa_b = s4.tile([128, HID], BF, bufs=1)
                        w1_pf_b = s4.tile([128, HID], BF, bufs=1)
                        _bcast_row(nc, w1_pa_b, w1_pa, HID)
                        _bcast_row(nc, w1_pf_b, w1_pf, HID)
                        asl_v = a2_out[:].rearrange("(k p) t -> p k t", p=128)
                        asl = [s4.tile([128, 16, 128], BF, name=f"asl{t}",
                                       bufs=1) for t in range(2)]
                        ao32 = [s4.tile([128, HID], F32, name=f"ao32_{t}",
                                        bufs=1) for t in range(2)]
                        xr = [s4.tile([128, HID], F32, tag="xr",
                                      name=f"xr{t}", bufs=2) for t in range(2)]
                        for t in range(2):
                            nc.sync.dma_start(asl[t][:],
                                              asl_v[:, :, t * 128:(t + 1) * 128])
                            nc.sync.dma_start(
                                xr[t][:], x_shard.ap()[t * 128:(t + 1) * 128, :])
                        st4 = [nrm.tile([128, 10, 6], F32, tag="st4",
                                        name=f"st4_{t}") for t in range(2)]
                        for n in range(10):
                            won = won_pre[n] if n < WOB else load_won(n)
                            for t in range(2):
                                pw = psP.tile([128, 256], F32, tag="mm", bufs=6,
                                              name=f"pw{t}_{n}")
                                for k in range(16):
                                    nc.tensor.matmul(pw[:], asl[t][:, k, :],
                                                     won[:, k, :],
                                                     start=(k == 0),
                                                     stop=(k == 15))
                                sl = slice(n * 256, (n + 1) * 256)
                                nc.scalar.activation(ao32[t][:, sl], pw[:],
                                                     AF.Copy)
                                nc.vector.bn_stats(out=st4[t][:, n, :],
                                                   in_=ao32[t][:, sl])
                        if taps:
                            for t in range(2):
                                nc.sync.dma_start(
                                    dbg_ao.ap()[t * 128:(t + 1) * 128, :],
                                    ao32[t][:])
                        rinv_as = [rinv_from_stats(st4[t][:], f"pan{t}")
                                   for t in range(2)]
                        x2s, h216s = [], []
                        for t in range(2):
                            x2 = ao32[t]  # in-place: ao32 becomes x2
                            nc.vector.scalar_tensor_tensor(
                                x2[:], ao32[t][:], rinv_as[t][:], w1_pa_b[:],
                                op0=ALU.mult, op1=ALU.mult)
                            nc.vector.tensor_add(x2[:], x2[:], xr[t][:])
                            nc.sync.dma_start(
                                x2_spill[t * 128:(t + 1) * 128, :], x2[:])
                            x2s.append(x2)
                        rinv_fs = [rmsnorm_rinv(x2s[t][:], HID, f"pff{t}")
                                   for t in range(2)]
                        for t in range(2):
                            h216 = s4.tile([128, HID], BF, tag="h216",
                                           name=f"h216_{t}", bufs=2)
                            nc.vector.scalar_tensor_tensor(
                                h216[:], x2s[t][:], rinv_fs[t][:], w1_pf_b[:],
                                op0=ALU.mult, op1=ALU.mult)
                            h216s.append(h216)
                            if taps:
                                nc.sync.dma_start(
                                    dbg_h2.ap()[t * 128:(t + 1) * 128, :],
                                    h216[:])
                                nc.sync.dma_start(
                                    dbg_x2.ap()[t * 128:(t + 1) * 128, :],
                                    x2s[t][:])
                        for k in range(KH):
                            for t in range(2):
                                ptr = psP.tile([128, 128], BF, tag="tr",
                                               bufs=2, name=f"s4t{t}{k}")
                                nc.tensor.transpose(
                                    ptr[:], h216s[t][:, k * 128:(k + 1) * 128],
                                    ident[:])
                                ts_ = slice(t * 128, (t + 1) * 128)
                                nc.scalar.activation(h2T8[:, k, 0, ts_], ptr[:],
                                                     AF.Copy)
                                nc.vector.tensor_sub(h2T8[:, k, 1, ts_], ptr[:],
                                                     h2T8[:, k, 0, ts_])

                    mark('S5')
                    # ============ S5: data-parallel MLP (C3 DR) ============
                    with tc.tile_pool(name="s5", bufs=2) as s5:
                        def h2_mov(k, kind):
                            if kind == "hi2":
                                return h2T8[:, k:k + 2, 0, :]
                            return _lohi(h2T8[:, k, 0:2, :], TS)

                        actT8 = s5.tile([128, MT, 2, TS], F8, bufs=1)
                        st6 = [nrm.tile([128, NT, 6], F32, tag="st6",
                                        name=f"st6_{t}") for t in range(2)]
                        for m in range(MT):
                            if m < WGB:
                                wgu = wgu_pre[m]
                            elif m < WGB + PF2:
                                wgu = pf2[m - WGB]
                            else:
                                wgu = load_wgu(m)
                            pg = psP.tile([128, TS], F32, tag="mm", bufs=6,
                                          name=f"pg{m}")
                            pu = psP.tile([128, TS], F32, tag="mm", bufs=6,
                                          name=f"pu{m}")

                            def wg_stat(k, kind, _w=wgu):
                                if kind == "hi2":
                                    return _w[:, 0, k:k + 2, 0, :]
                                return _w[:, 0, k, 0:2, :]

                            def wu_stat(k, kind, _w=wgu):
                                if kind == "hi2":
                                    return _w[:, 1, k:k + 2, 0, :]
                                return _w[:, 1, k, 0:2, :]

                            for i in range(KP):
                                c3_matmuls(pg[:], wg_stat, h2_mov, i,
                                           i == 0, i == KP - 1)
                                c3_matmuls(pu[:], wu_stat, h2_mov, i,
                                           i == 0, i == KP - 1)
                            gsc = s5.tile([128, TS], F32, tag="gsc",
                                          name=f"gsc{m}", bufs=2)
                            nc.scalar.activation(gsc[:], pg[:],
                                                 AF.Gelu_apprx_tanh,
                                                 scale=1.0 / WSC)
                            gu32 = s5.tile([128, TS], F32, tag="gu32",
                                           name=f"gu32_{m}", bufs=2)
                            nc.vector.scalar_tensor_tensor(
                                gu32[:], pu[:], 1.0 / WSC, gsc[:],
                                op0=ALU.mult, op1=ALU.mult)
                            if taps and m == 0:
                                nc.sync.dma_start(dbg_gu.ap(), gu32[:])
                            nc.vector.tensor_copy(actT8[:, m, 0, :], gu32[:])
                            nc.vector.tensor_sub(actT8[:, m, 1, :], gu32[:],
                                                 actT8[:, m, 0, :])

                        def act_mov(j, kind):
                            if kind == "hi2":
                                return actT8[:, j:j + 2, 0, :]
                            return _lohi(actT8[:, j, 0:2, :], TS)

                        for n in range(NT):
                            wdn = s5.tile([128, MT, 128], F8, tag="wdn",
                                          name=f"wdn{n}", bufs=2)
                            nc.gpsimd.dma_start(
                                wdn[:], wd8.ap()[n * 128:(n + 1) * 128, :]
                                .rearrange("p (j c) -> p j c", j=MT))
                            pd = psP.tile([128, TS], F32, tag="mm", bufs=6,
                                          name=f"pd{n}")
                            # down: plain-fp8 weights, hi/lo-compensated acts
                            # (2 DR per pair-chunk, both pairing across chunks)
                            for j in range(JP):
                                a, b = 2 * j, 2 * j + 1
                                nc.tensor.matmul(
                                    pd[:], wdn[:, a:a + 2, :],
                                    actT8[:, a:a + 2, 0, :],
                                    start=(j == 0), stop=False, perf_mode=DR)
                                nc.tensor.matmul(
                                    pd[:], wdn[:, a:a + 2, :],
                                    actT8[:, a:a + 2, 1, :],
                                    start=False, stop=(j == JP - 1),
                                    perf_mode=DR)
                            md = s5.tile([128, TS], BF, tag="md",
                                         name=f"md{n}", bufs=3)
                            nc.scalar.activation(md[:], pd[:], AF.Copy,
                                                 scale=1.0 / WSC)
                            for t in range(2):
                                ptr = psP.tile([128, 128], BF, tag="tr", bufs=2,
                                               name=f"s5t{n}_{t}")
                                nc.tensor.transpose(
                                    ptr[:], md[:, t * 128:(t + 1) * 128],
                                    ident[:])
                                nc.vector.tensor_copy(
                                    mlp16[t][:, n * 128:(n + 1) * 128], ptr[:])
                                nc.vector.bn_stats(
                                    out=st6[t][:, n, :],
                                    in_=mlp16[t][:, n * 128:(n + 1) * 128])

                    mark('S6')
                    # ============ S6: post_ff norm + residual ============
                    with tc.tile_pool(name="s6", bufs=2) as s6:
                        for t in range(2):
                            if taps:
                                nc.sync.dma_start(
                                    dbg_mlp.ap()[t * 128:(t + 1) * 128, :],
                                    mlp16[t][:])
                            rinv_o = rinv_from_stats(st6[t][:], f"pon{t}")
                            o32 = s6.tile([128, HID], F32, tag="o32",
                                          name=f"o32_{t}", bufs=2)
                            nc.vector.scalar_tensor_tensor(
                                o32[:], mlp16[t][:], rinv_o[:], w1_po_b[:],
                                op0=ALU.mult, op1=ALU.mult)
                            x2l = s6.tile([128, HID], F32, tag="x2l",
                                          name=f"x2l{t}", bufs=2)
                            nc.sync.dma_start(
                                x2l[:], x2_spill[t * 128:(t + 1) * 128, :])
                            nc.vector.tensor_add(o32[:], o32[:], x2l[:])
                            nc.sync.dma_start(
                                out_shard.ap()[t * 128:(t + 1) * 128, :], o32[:])

    nc.compile()
    return nc


_NC_CACHE = None


def _get_nc():
    global _NC_CACHE
    if _NC_CACHE is None:
        _NC_CACHE = build_nc()
    return _NC_CACHE


def _hi_lo(w):
    f8 = ml_dtypes.float8_e4m3
    hi = w.astype(f8)
    lo = (w - hi.astype(np.float32)).astype(f8)
    return hi, lo


def _pack_pairs(w, kchunks, ntiles):
    """w [kchunks*128, ntiles*128*cw] f32 -> [ntiles*128, kchunks*2*cw] f8
    with row n*128+p, col k*(2cw)+hl*cw+c = hl-part of w[k*128+p, n*cw+c]."""
    K, N = w.shape
    cw = N // ntiles
    hi, lo = _hi_lo(w)
    st = np.stack([hi, lo], 0).reshape(2, kchunks, 128, ntiles, cw)
    return np.ascontiguousarray(
        st.transpose(3, 2, 1, 0, 4).reshape(ntiles * 128, kchunks * 2 * cw))


def make_in_maps(hidden_states, position_ids, wq, wk, wv, wo, q_ln_w, k_ln_w,
                 in_ln_w, post_attn_ln_w, pre_ff_ln_w, post_ff_ln_w,
                 w_gate, w_up, w_down):
    bf16 = ml_dtypes.bfloat16
    f32 = np.float32
    x = np.asarray(hidden_states, f32).reshape(S, HID)
    pos = np.asarray(position_ids).reshape(S).astype(np.float64)

    inv_freq = 1.0 / (BASE ** (np.arange(0, HD, 2, dtype=np.float64) / HD))
    freqs = pos[:, None] * inv_freq[None, :]
    emb = np.concatenate([freqs, freqs], axis=1)
    cos = np.cos(emb).astype(f32)
    sin = np.sin(emb).astype(f32)
    w1q = 1.0 + np.asarray(q_ln_w, f32)
    w1k = 1.0 + np.asarray(k_ln_w, f32)

    def rope_tabs(w1):
        w1sw = np.concatenate([w1[HALF:], w1[:HALF]])
        sgn = np.concatenate([-np.ones(HALF, f32), np.ones(HALF, f32)])
        return ((cos * w1[None, :]).astype(bf16),
                (sin * (w1sw * sgn)[None, :]).astype(bf16))

    cqw_np, sqw_np = rope_tabs(w1q)
    ckw_np, skw_np = rope_tabs(w1k)

    wqkv = np.concatenate([np.asarray(wq, f32), np.asarray(wk, f32),
                           np.asarray(wv, f32)], axis=1) * WSC
    wqkv8_np = _pack_pairs(wqkv, KH, ND)          # chunk width 256
    wgu8_np = np.ascontiguousarray(np.concatenate(
        [_pack_pairs(np.asarray(w_gate, f32) * WSC, KH, MT),
         _pack_pairs(np.asarray(w_up, f32) * WSC, KH, MT)], axis=1))
    wd_s = np.asarray(w_down, f32) * WSC
    wd8_np = np.ascontiguousarray(
        wd_s.reshape(MT, 128, NT, 128).transpose(2, 1, 0, 3)
        .reshape(NT * 128, MT * 128).astype(ml_dtypes.float8_e4m3))

    common = {
        "wqkv8": wqkv8_np,
        "wo16": np.asarray(wo, f32).astype(bf16),
        "wgu8": wgu8_np, "wd8": wd8_np,
        "w1_in": (1.0 + np.asarray(in_ln_w, f32)).astype(bf16),
        "w1_pa": (1.0 + np.asarray(post_attn_ln_w, f32)).astype(bf16),
        "w1_pf": (1.0 + np.asarray(pre_ff_ln_w, f32)).astype(bf16),
        "w1_po": 1.0 + np.asarray(post_ff_ln_w, f32),
    }
    in_maps = []
    for c in range(NC_):
        sl = slice(c * TS, (c + 1) * TS)
        in_maps.append({
            "x_shard": np.ascontiguousarray(x[sl]),
            "cqw": np.ascontiguousarray(cqw_np[sl]),
            "sqw": np.ascontiguousarray(sqw_np[sl]),
            "ckw": np.ascontiguousarray(ckw_np[sl]),
            "skw": np.ascontiguousarray(skw_np[sl]),
            **common,
        })
    return in_maps


def kernel(**inputs):
    in_maps = make_in_maps(**inputs)
    nc = _get_nc()
    res = run_bass_kernel_spmd(nc, in_maps, core_ids=list(range(NC_)))
    out = np.concatenate([res.results[c]["out_shard"] for c in range(NC_)],
                         axis=0)
    return out.reshape(1, S, HID).astype(np.float32)
